# revision 85
# baseline (speedup 1.0000x reference)
"""Trainium2 Bass kernel for dense-transformer attention block.

Reference computation (see harness):
  xn  = x / max(||x||_2, 1e-12) * sqrt(dim) * gamma          (RMSNorm-as-written)
  q   = (xn @ Wq.T) * dh^-0.5 ; k, v = split(xn @ Wkv.T)
  sim = q k^T + attn_bias ; key-pad mask ; causal mask
  out = softmax(sim) @ v @ Wo.T

Sharding: 16 heads / 8 cores = 2 heads per core (tensor parallel).
Each core computes its 2 heads' attention + its column-slice of Wo,
producing a partial output; host sums the 8 partials.

Device dataflow (per core), everything in transposed token-on-free layout:
  qT/kT/vT = W^T-stationary matmuls over xnT (bf16, N=512)
  v        = PE-transpose of vT, + ones column (softmax denominator trick)
  S^T      = kT.T-slices @ qT  (per head, causal-triangular blocks only)
  E        = exp(S^T) * exp_bias_T   (exp(bias) precomputed on host, causal-
             masked there; softmax needs no max-subtraction: |logits| < ~15)
  O^T      = v' stationary @ E  -> row 64 = denominator
  o2       = O^T[0:64] * broadcast(1/denom)   (folded into the po evacuation)
  out^T    = WoT-chunk stationary @ o2
Host prep: RMSNorm + transposes + weight folding + exp(bias) (elementwise);
all GEMMs and softmax run on device. All HBM traffic is bf16.
"""
import sys
import numpy as np

sys.path.insert(0, "/opt/trn_rl_repo")

import ml_dtypes  # noqa: E402

N_CORES = 8
B = 2
N = 2048
DIM = 1024
HEADS = 16
DH = 64
H_LOC = HEADS // N_CORES  # 2 heads per core
NT = N // 128             # 16 token tiles of 128
NIR = N // 512            # 4 i-ranges of 512
NC_CHUNK = DIM // 128     # 8 contraction chunks

_BUILT = {}


def _build():
    """Construct + compile the per-core Bass program (same for all cores)."""
    import concourse.bass as bass
    import concourse.mybir as mybir
    from concourse import bacc
    from concourse.tile import TileContext
    from concourse.masks import make_identity

    f32 = mybir.dt.float32
    bf16 = mybir.dt.bfloat16
    f8 = mybir.dt.float8e4

    nc = bacc.Bacc("TRN2", target_bir_lowering=False, debug=False,
                   num_devices=N_CORES)

    xnT = nc.dram_tensor("xnT", [B, DIM, N], f8, kind="ExternalInput").ap()
    xnTc = nc.dram_tensor("xnTc", [B, DIM, 128], bf16, kind="ExternalInput").ap()
    # q/k/v weights in fp8 DoubleRow SBUF layout (x32); wvc = bf16 x32 copy
    # of wv for the tokens 0-127 correction patch (early causal rows see few
    # keys, so v quantization error there passes straight to the output)
    wq = nc.dram_tensor("wq", [128, DIM], f8, kind="ExternalInput").ap()
    wk = nc.dram_tensor("wk", [128, DIM], f8, kind="ExternalInput").ap()
    wv = nc.dram_tensor("wv", [128, DIM], f8, kind="ExternalInput").ap()
    wvc = nc.dram_tensor("wvc", [128, DIM], bf16, kind="ExternalInput").ap()
    wo = nc.dram_tensor("wo", [128, DIM], bf16, kind="ExternalInput").ap()
    eb = nc.dram_tensor("eb", [H_LOC, N, N], bf16, kind="ExternalInput").ap()
    o = nc.dram_tensor("o", [B, DIM, N], bf16, kind="ExternalOutput").ap()

    with TileContext(nc) as tc:
        with (
            tc.tile_pool(name="const", bufs=1) as constp,
            tc.tile_pool(name="wts", bufs=1) as wtp,
            tc.tile_pool(name="xn", bufs=4) as xnp,
            tc.tile_pool(name="qkv", bufs=1) as qkvp,
            tc.tile_pool(name="ebp", bufs=40) as ebpool,
            tc.tile_pool(name="ework", bufs=4) as ework,
            tc.tile_pool(name="o2", bufs=1) as o2p,
            tc.tile_pool(name="dance", bufs=3) as dancep,
            tc.tile_pool(name="outs", bufs=9) as outsp,
            tc.tile_pool(name="qkh", bufs=2, space="PSUM") as psA,
            tc.tile_pool(name="mmhalf", bufs=2, space="PSUM") as psD,
            tc.tile_pool(name="pacc", bufs=2, space="PSUM") as psB,
        ):
            ident = constp.tile([128, 128], f32)
            make_identity(nc, ident[:])
            identb = constp.tile([128, 128], bf16, name="identb")
            nc.vector.tensor_copy(identb[:], ident[:])

            wq_sb = wtp.tile([128, NC_CHUNK * 128], f8, tag="w1")
            wk_sb = wtp.tile([128, NC_CHUNK * 128], f8, tag="wk")
            wv_sb = wtp.tile([128, NC_CHUNK * 128], f8, tag="wv")
            wvc_sb = wtp.tile([128, NC_CHUNK * 128], bf16, tag="wvc")
            wo_sb = wtp.tile([128, DIM], bf16, tag="wo", name="wo_sb")
            # load order matches first use: v, k, q (proj runs v first)
            for wsb_, wdr_ in ((wv_sb, wv), (wk_sb, wk), (wq_sb, wq),
                               (wo_sb, wo), (wvc_sb, wvc)):
                nc.scalar.dma_start(out=wsb_[:], in_=wdr_[:])

            qT = [qkvp.tile([128, N], bf16, tag=f"qT{b}", name=f"qT{b}")
                  for b in range(B)]
            kT = [qkvp.tile([128, N], bf16, tag=f"kT{b}", name=f"kT{b}")
                  for b in range(B)]
            vT = [qkvp.tile([128, N], bf16, tag=f"vT{b}", name=f"vT{b}")
                  for b in range(B)]
            vn = [qkvp.tile([128, H_LOC * NT * 65], bf16, tag=f"vn{b}",
                            name=f"vn{b}") for b in range(B)]
            o2 = [o2p.tile([128, N], bf16, tag=f"o2{b}", name=f"o2{b}")
                  for b in range(B)]
            # ones columns of vn (every 65th col), written once per batch
            for b in range(B):
                nc.vector.memset(
                    vn[b][:].rearrange("p (t c) -> p t c", c=65)[:, :, 64:65],
                    1.0)

            def gen_proj(b, evac_dve=False):
                # all projections via fp8 DoubleRow (4 chunk-pairs, x32
                # weights); v gets a bf16 correction for tokens 0-127
                for irp in range(NIR // 2):
                    if b == 0 and irp == 0:
                        x8t = [xnp.tile([128, 2 * 1024], f8, tag="x8", bufs=4,
                                        name=f"x8{b}_{irp}_{g}")
                               for g in range(4)]
                        x8 = [t[:] for t in x8t]
                        for g in range(4):
                            eng = nc.scalar if g % 2 == 1 else nc.sync
                            eng.dma_start(
                                out=x8[g].rearrange("p (kt f) -> p kt f",
                                                    kt=2),
                                in_=xnT[b, g * 256:(g + 1) * 256,
                                        irp * 1024:(irp + 1) * 1024].rearrange(
                                            "(kt p) f -> p kt f", kt=2))
                    else:
                        xw = xnp.tile([128, 8 * 1024], f8, tag="x8w", bufs=2,
                                      name=f"x8w{b}_{irp}")
                        nc.sync.dma_start(
                            out=xw[:].rearrange("p (g kt f) -> p g kt f",
                                                g=4, kt=2),
                            in_=xnT[b, :, irp * 1024:(irp + 1) * 1024].rearrange(
                                "(g kt p) f -> p g kt f", g=4, kt=2))
                        x8 = [xw[:, g * 2048:(g + 1) * 2048] for g in range(4)]
                    if irp == 0:
                        xc = xnp.tile([128, NC_CHUNK * 128], bf16, tag="xc",
                                      bufs=2, name=f"xc{b}")
                        nc.sync.dma_start(
                            out=xc[:].rearrange("p (c f) -> p c f", c=NC_CHUNK),
                            in_=xnTc[b].rearrange("(c p) f -> p c f",
                                                  c=NC_CHUNK))
                    for wsb, dst in ((wv_sb, vT[b]), (wk_sb, kT[b]),
                                     (wq_sb, qT[b])):
                        wsb4 = wsb[:].rearrange("p (g kt j) -> p g kt j",
                                                g=4, kt=2)
                        for half in range(2):
                            p = psD.tile([128, 512], f32, tag="mm",
                                         name=f"pp{b}_{irp}_{half}")
                            for g in range(4):
                                nc.tensor.matmul(
                                    p[:],
                                    wsb4[:, g],
                                    x8[g].rearrange(
                                        "p (kt f) -> p kt f", kt=2)[
                                        :, :, half * 512:(half + 1) * 512],
                                    start=(g == 0), stop=(g == 3),
                                    perf_mode=mybir.MatmulPerfMode.DoubleRow)
                            sl = dst[:, irp * 1024 + half * 512:
                                     irp * 1024 + (half + 1) * 512]
                            if (evac_dve or irp == 1) and half == 0:
                                nc.vector.tensor_copy(sl, p[:])
                            else:
                                nc.scalar.copy(sl, p[:])
                        if wsb is wv_sb and irp == 0:
                            # bf16 recompute of v for tokens 0-127
                            pc = psD.tile([128, 128], f32, tag="mm",
                                          name=f"pvc{b}")
                            for c in range(NC_CHUNK):
                                nc.tensor.matmul(
                                    pc[:],
                                    wvc_sb[:, c * 128:(c + 1) * 128],
                                    xc[:, c * 128:(c + 1) * 128],
                                    start=(c == 0), stop=(c == NC_CHUNK - 1))
                            if evac_dve:
                                nc.vector.tensor_copy(vT[b][:, 0:128], pc[:])
                            else:
                                nc.scalar.copy(vT[b][:, 0:128], pc[:])
                        yield

            def gen_vt(b):
                for jt in range(NT):
                    pv = psD.tile([128, 128], bf16, tag="mm",
                                  name=f"pv{b}_{jt}")
                    nc.tensor.matmul(pv[:], vT[b][:, jt * 128:(jt + 1) * 128],
                                     identb[:], is_transpose=True)
                    for h in range(H_LOC):
                        base = (h * NT + jt) * 65
                        if b == 0 and jt < 8:
                            nc.scalar.copy(vn[b][:, base:base + 64],
                                           pv[:, h * 64:h * 64 + 64])
                        else:
                            nc.vector.tensor_copy(vn[b][:, base:base + 64],
                                                  pv[:, h * 64:h * 64 + 64])
                    if jt % 4 == 3:
                        yield

            def gen_wo(b, irp, evac_dve=True):
                # per-(half, ec) steps, half-major; stage-pipelined evac;
                # one output DMA per ec (fires after the half-1 evacuation)
                pend = None
                osb_tiles = {}

                def evac(p):
                    pw, ec, half, i = p
                    if ec not in osb_tiles:
                        osb_tiles[ec] = outsp.tile(
                            [128, 1024], bf16, tag="osb",
                            name=f"osb{b}_{ec}_{irp}")
                    osb = osb_tiles[ec]
                    dst = osb[:, half * 512:(half + 1) * 512]
                    if evac_dve is True or (evac_dve == "alt" and (ec + half) % 2 == 0):
                        nc.vector.tensor_copy(dst, pw[:])
                    else:
                        nc.scalar.copy(dst, pw[:])
                    if half == 1:
                        nc.sync.dma_start(
                            out=o[b, ec * 128:(ec + 1) * 128,
                                  irp * 1024:(irp + 1) * 1024],
                            in_=osb[:])

                i = 0
                for half in range(2):
                    for ec in range(NC_CHUNK):
                        pw = psD.tile([128, 512], f32, tag="mm",
                                      name=f"pw{b}_{ec}_{irp}_{half}")
                        nc.tensor.matmul(
                            pw[:],
                            wo_sb[:, ec * 128:(ec + 1) * 128],
                            o2[b][:, irp * 1024 + half * 512:
                                  irp * 1024 + (half + 1) * 512],
                            start=True, stop=True)
                        if pend is not None:
                            evac(pend)
                        pend = (pw, ec, half, i)
                        i += 1
                        yield
                evac(pend)

            eb_tiles = {}

            def emit_eb_ir(h, ir):
                if (h, ir) in eb_tiles:
                    return
                njt = min(4 * ir + 4, NT)
                t = ebpool.tile([128, njt * 512], bf16, tag=f"eb{ir}", bufs=2,
                                name=f"eb_{h}_{ir}")
                nc.sync.dma_start(
                    out=t[:].rearrange("p (j f) -> p j f", j=njt),
                    in_=eb[h, 0:njt * 128, ir * 512:(ir + 1) * 512].rearrange(
                        "(j p) f -> p j f", j=njt))
                eb_tiles[(h, ir)] = t

            def emit_eb_tile(h, jtp, ir):
                emit_eb_ir(h, ir)

            def gen_eb(h):
                for ir in range(NIR):
                    emit_eb_ir(h, ir)
                    yield

            def gen_attn(h, b):
                # stage-pipelined: PV (+ epilogue at ir close) for block k-1
                # is emitted after QK/exp/mul of block k, so no engine queue
                # entry ever waits long (avoids head-of-line blocking).
                pends = []

                def epilogue(po, ir):
                    rfl = dancep.tile([1, 512], bf16, tag="rfl", bufs=2,
                                      name=f"rf{h}_{b}_{ir}")
                    with nc.allow_low_precision(reason="bf16 1/denom"):
                        nc.vector.reciprocal(rfl[0:1, :], po[64:65, :])
                    rb = ework.tile([128, 512], bf16, tag="rb", bufs=2,
                                    name=f"rb{h}_{b}_{ir}")
                    for s in range(4):
                        nc.gpsimd.partition_broadcast(
                            rb[:, s * 128:(s + 1) * 128],
                            rfl[0:1, s * 128:(s + 1) * 128])
                    if h == 0:
                        nc.vector.tensor_mul(
                            o2[b][0:64, ir * 512:(ir + 1) * 512],
                            po[0:64, :], rb[0:64, :])
                    else:
                        tmp = ework.tile([64, 512], bf16, tag="hshift",
                                         bufs=2, name=f"hs{h}_{b}_{ir}")
                        nc.vector.tensor_mul(tmp[:], po[0:64, :], rb[0:64, :])
                        nc.sync.dma_start(
                            out=o2[b][64:128, ir * 512:(ir + 1) * 512],
                            in_=tmp[:])

                def stage2(p):
                    e2, jtp, jt_max, po, ir = p
                    for half in range(2):
                        jt = 2 * jtp + half
                        vbase = (h * NT + jt) * 65
                        nc.tensor.matmul(
                            po[:], vn[b][:, vbase:vbase + 65],
                            e2[:, half * 512:(half + 1) * 512],
                            start=(jt == 0), stop=(jt == jt_max - 1))
                    if 2 * jtp + 1 == jt_max - 1:
                        epilogue(po, ir)

                for ir in range(NIR):
                    jt_max = min(4 * ir + 4, NT)
                    po = psB.tile([65, 512], f32, tag="po",
                                  name=f"po{h}_{b}_{ir}")
                    for jtp in range(jt_max // 2):
                        emit_eb_ir(h, ir)
                        ebt = eb_tiles[(h, ir)][:, jtp * 1024:(jtp + 1) * 1024]
                        ps = psA.tile([128, 1024], f32, tag="qk",
                                      name=f"ps{h}_{b}_{ir}_{jtp}")
                        for half in range(2):
                            jt = 2 * jtp + half
                            nc.tensor.matmul(
                                ps[:, half * 512:(half + 1) * 512],
                                kT[b][h * 64:(h + 1) * 64,
                                      jt * 128:(jt + 1) * 128],
                                qT[b][h * 64:(h + 1) * 64,
                                      ir * 512:(ir + 1) * 512],
                                start=True, stop=True)
                        e1 = ework.tile([128, 1024], bf16, tag="e1", bufs=6,
                                        name=f"e1_{h}_{b}_{ir}_{jtp}")
                        nc.scalar.activation(e1[:], ps[:],
                                             mybir.ActivationFunctionType.Exp,
                                             scale=1.0 / 1024.0)
                        e2 = ework.tile([128, 1024], bf16, tag="e2", bufs=6,
                                        name=f"e2_{h}_{b}_{ir}_{jtp}")
                        nc.vector.tensor_mul(e2[:], e1[:], ebt)
                        pends.append((e2, jtp, jt_max, po, ir))
                        if len(pends) > 1:
                            stage2(pends.pop(0))
                        yield
                for p_ in pends:
                    stage2(p_)
                pends.clear()

            def run(g):
                for _ in g:
                    pass

            def take(g, n):
                for _ in range(n):
                    try:
                        next(g)
                    except StopIteration:
                        return False
                return True

            def rr2(main, other, ratio=2):
                """Interleave ratio:1; stop when main exhausts (other may
                have leftovers for a later phase)."""
                while True:
                    for _ in range(ratio):
                        if not take(main, 1):
                            return
                    take(other, 1)

            def chain_steps(steps):
                for g, n in steps:
                    for _ in range(n):
                        try:
                            yield next(g)
                        except StopIteration:
                            break
                for g, _ in steps:
                    yield from g

            def zip_gens(*pairs):
                """Round-robin over (gen, per-slot count); yield per round."""
                active = [[g, c] for g, c in pairs]
                while active:
                    for it in list(active):
                        for _ in range(it[1]):
                            try:
                                next(it[0])
                            except StopIteration:
                                active.remove(it)
                                break
                        yield

            def pair(ga, gb):
                """Strict block-by-block interleave of two attention streams."""
                while True:
                    a_ok = take(ga, 1)
                    b_ok = take(gb, 1)
                    if not (a_ok or b_ok):
                        return
                    yield

            p0 = gen_proj(0)
            vt0 = gen_vt(0)
            p1 = gen_proj(1, evac_dve=True)
            v1 = gen_vt(1)
            ebz = chain_steps([(gen_eb(0), 100), (gen_eb(1), 100)])

            # intro: proj-b0 irp0 (v,k,q) + vt jt0-7, eb-h0 trickling on DMA
            for _ in range(3):
                take(p0, 1)
                take(ebz, 1)
            take(vt0, 1)

            # phase C: both b0 attention streams paired, over the rest of
            # b0-proj, all of b1-proj, and wo(b0,0) once deps complete
            paC = pair(gen_attn(0, 0), gen_attn(1, 0))
            bgC = chain_steps([(vt0, 1), (p0, 3), (vt0, 2), (p1, 6), (v1, 4)])
            for _ in range(7):
                take(paC, 1)
                take(bgC, 1)
            w00 = gen_wo(0, 0, evac_dve="alt")
            bgC2 = chain_steps([(bgC, 999), (w00, 100), (ebz, 999)])
            rr2(paC, bgC2, 1)
            run(bgC)   # safety barrier: all projections emitted
            run(w00)

            # phase E: both b1 attention streams over remaining wo work.
            paE = pair(gen_attn(0, 1), gen_attn(1, 1))
            w01 = gen_wo(0, 1, evac_dve="alt")    # deps met (phase C done)
            ww = chain_steps([(w01, 100)])
            for _ in range(14):
                take(paE, 1)
                take(ww, 1)
                if _ >= 7 and _ % 2 == 1:
                    take(ww, 1)
            run(w01)
            w10 = gen_wo(1, 0, evac_dve="alt")    # ir0-1 epilogues done
            for _ in range(4):
                take(paE, 1)
                take(w10, 4)
            run(w10)
            w11 = gen_wo(1, 1, evac_dve="alt")    # half0 needs ir2 (round ~14)
            rr2(paE, chain_steps([(w11, 8), (ebz, 999)]), 1)
            run(w11)

    nc.compile()
    return nc


def _get_nc():
    if "nc" not in _BUILT:
        _BUILT["nc"] = _build()
    return _BUILT["nc"]


def _reference_np(x, attn_bias, mask, gamma, Wq, Wkv, Wo):
    """Numpy fallback (only used if mask is not all-True)."""
    b, n, dim = x.shape
    h, dh = HEADS, DH
    l2 = np.sqrt((x.astype(np.float64) ** 2).sum(-1, keepdims=True))
    xn = x / np.maximum(l2, 1e-12) * (dim ** 0.5) * gamma
    q = (xn @ Wq.T) * (dh ** -0.5)
    kv = xn @ Wkv.T
    k, v = kv[..., :h * dh], kv[..., h * dh:]
    def to_heads(t):
        return t.reshape(b, n, h, dh).transpose(0, 2, 1, 3)
    q, k, v = to_heads(q), to_heads(k), to_heads(v)
    sim = np.einsum('bhid,bhjd->bhij', q, k) + attn_bias
    neg = -np.finfo(np.float32).max
    sim = np.where(mask[:, None, None, :], sim, neg)
    causal = np.triu(np.ones((n, n), dtype=bool), k=1)
    sim = np.where(causal, neg, sim)
    sim = sim - sim.max(-1, keepdims=True)
    e = np.exp(sim)
    attn = e / e.sum(-1, keepdims=True)
    out = np.einsum('bhij,bhjd->bhid', attn, v)
    out = out.transpose(0, 2, 1, 3).reshape(b, n, h * dh)
    return (out @ Wo.T).astype(np.float32)


def kernel(x, attn_bias, mask, gamma, Wq, Wkv, Wo, _trace=False):
    from concourse.bass_utils import run_bass_kernel_spmd

    x = np.asarray(x, dtype=np.float32)
    attn_bias = np.asarray(attn_bias, dtype=np.float32)
    mask = np.asarray(mask)
    gamma = np.asarray(gamma, dtype=np.float32)
    Wq = np.asarray(Wq, dtype=np.float32)
    Wkv = np.asarray(Wkv, dtype=np.float32)
    Wo = np.asarray(Wo, dtype=np.float32)

    if not bool(mask.all()):
        return _reference_np(x, attn_bias, mask, gamma, Wq, Wkv, Wo)

    # ---- host prep (elementwise / layout only) ----
    bf = ml_dtypes.bfloat16
    f8 = ml_dtypes.float8_e4m3
    WS = 32.0   # fp8 weight scale; q,k each carry x32 -> exp scale 1/1024,
                # v carries x32 -> folded into Wo below
    l2 = np.sqrt((x ** 2).sum(-1, keepdims=True))
    xn = x / np.maximum(l2, 1e-12) * (DIM ** 0.5) * gamma
    xnT = np.ascontiguousarray(xn.transpose(0, 2, 1))            # [B, DIM, N]
    xnT8 = xnT.astype(f8)
    xnTc = np.ascontiguousarray(xnT[:, :, :128]).astype(bf)

    tril = np.tril(np.ones((N, N), dtype=np.float32))

    def f8_layout(wT):
        """[DIM,128] -> [128, DIM] DoubleRow image:
        out[p, ((g*2 + kt)*128) + j] = wT[g*256 + kt*128 + p, j]."""
        return np.ascontiguousarray(
            wT.reshape(4, 2, 128, 128).transpose(2, 0, 1, 3).reshape(128, DIM))

    def sbuf_layout(wT):
        """[DIM,128] -> [128, DIM] SBUF image: out[p, c*128+j] = wT[c*128+p, j]."""
        return np.ascontiguousarray(
            wT.reshape(NC_CHUNK, 128, 128).transpose(1, 0, 2).reshape(128, DIM))

    in_maps = []
    for c in range(N_CORES):
        r0 = c * 128
        wq_c = f8_layout((Wq[r0:r0 + 128] * (WS * DH ** -0.5)).T).astype(f8)
        wk_c = f8_layout((Wkv[r0:r0 + 128] * WS).T).astype(f8)
        wv_c = f8_layout(
            (Wkv[HEADS * DH + r0:HEADS * DH + r0 + 128] * WS).T).astype(f8)
        wvc_c = sbuf_layout(
            (Wkv[HEADS * DH + r0:HEADS * DH + r0 + 128] * WS).T).astype(bf)
        wo_c = np.ascontiguousarray(
            (Wo[:, r0:r0 + 128] / WS).T).astype(bf)  # [128, DIM]
        bias_c = attn_bias[H_LOC * c:H_LOC * (c + 1)]            # [2, N, N]
        ebc = np.exp(bias_c) * tril                              # mask j>i
        ebc = np.ascontiguousarray(ebc.transpose(0, 2, 1)).astype(bf)  # [h,j,i]
        in_maps.append({"xnT": xnT8, "xnTc": xnTc, "wq": wq_c, "wk": wk_c,
                        "wv": wv_c, "wvc": wvc_c, "wo": wo_c, "eb": ebc})

    nc = _get_nc()
    try:
        res = run_bass_kernel_spmd(nc, in_maps,
                                   core_ids=list(range(N_CORES)),
                                   trace=_trace)
    except ModuleNotFoundError:
        # NTFF profiling hook unavailable in this environment
        res = run_bass_kernel_spmd(nc, in_maps,
                                   core_ids=list(range(N_CORES)))
    acc = res.results[0]["o"].astype(np.float32)
    for c in range(1, N_CORES):
        acc += res.results[c]["o"].astype(np.float32)
    out = np.ascontiguousarray(acc.transpose(0, 2, 1))           # [B, N, DIM]
    if _trace:
        kernel._last_results = res
    return out


# revision 86
# speedup vs baseline: 1.0165x; 1.0165x over previous
"""Trainium2 Bass kernel for dense-transformer attention block.

Reference computation (see harness):
  xn  = x / max(||x||_2, 1e-12) * sqrt(dim) * gamma          (RMSNorm-as-written)
  q   = (xn @ Wq.T) * dh^-0.5 ; k, v = split(xn @ Wkv.T)
  sim = q k^T + attn_bias ; key-pad mask ; causal mask
  out = softmax(sim) @ v @ Wo.T

Sharding: 16 heads / 8 cores = 2 heads per core (tensor parallel).
Each core computes its 2 heads' attention + its column-slice of Wo,
producing a partial output; host sums the 8 partials.

Device dataflow (per core), everything in transposed token-on-free layout:
  qT/kT/vT = W^T-stationary matmuls over xnT (bf16, N=512)
  v        = PE-transpose of vT, + ones column (softmax denominator trick)
  S^T      = kT.T-slices @ qT  (per head, causal-triangular blocks only)
  E        = exp(S^T) * exp_bias_T   (exp(bias) precomputed on host, causal-
             masked there; softmax needs no max-subtraction: |logits| < ~15)
  O^T      = v' stationary @ E  -> row 64 = denominator
  o2       = O^T[0:64] * broadcast(1/denom)   (folded into the po evacuation)
  out^T    = WoT-chunk stationary @ o2
Host prep: RMSNorm + transposes + weight folding + exp(bias) (elementwise);
all GEMMs and softmax run on device. All HBM traffic is bf16.
"""
import sys
import numpy as np

sys.path.insert(0, "/opt/trn_rl_repo")

import ml_dtypes  # noqa: E402

N_CORES = 8
B = 2
N = 2048
DIM = 1024
HEADS = 16
DH = 64
H_LOC = HEADS // N_CORES  # 2 heads per core
NT = N // 128             # 16 token tiles of 128
NIR = N // 512            # 4 i-ranges of 512
NC_CHUNK = DIM // 128     # 8 contraction chunks

_BUILT = {}


def _build():
    """Construct + compile the per-core Bass program (same for all cores)."""
    import concourse.bass as bass
    import concourse.mybir as mybir
    from concourse import bacc
    from concourse.tile import TileContext
    from concourse.masks import make_identity

    f32 = mybir.dt.float32
    bf16 = mybir.dt.bfloat16
    f8 = mybir.dt.float8e4

    nc = bacc.Bacc("TRN2", target_bir_lowering=False, debug=False,
                   num_devices=N_CORES)

    xnT = nc.dram_tensor("xnT", [B, DIM, N], f8, kind="ExternalInput").ap()
    xnTc = nc.dram_tensor("xnTc", [B, DIM, 128], bf16, kind="ExternalInput").ap()
    # q/k/v weights in fp8 DoubleRow SBUF layout (x32); wvc = bf16 x32 copy
    # of wv for the tokens 0-127 correction patch (early causal rows see few
    # keys, so v quantization error there passes straight to the output)
    wq = nc.dram_tensor("wq", [128, DIM], f8, kind="ExternalInput").ap()
    wk = nc.dram_tensor("wk", [128, DIM], f8, kind="ExternalInput").ap()
    wv = nc.dram_tensor("wv", [128, DIM], f8, kind="ExternalInput").ap()
    wvc = nc.dram_tensor("wvc", [128, DIM], bf16, kind="ExternalInput").ap()
    wo = nc.dram_tensor("wo", [128, DIM], bf16, kind="ExternalInput").ap()
    eb = nc.dram_tensor("eb", [H_LOC, N, N], bf16, kind="ExternalInput").ap()
    o = nc.dram_tensor("o", [B, DIM, N], bf16, kind="ExternalOutput").ap()

    with TileContext(nc) as tc:
        with (
            tc.tile_pool(name="const", bufs=1) as constp,
            tc.tile_pool(name="wts", bufs=1) as wtp,
            tc.tile_pool(name="xn", bufs=4) as xnp,
            tc.tile_pool(name="qkv", bufs=1) as qkvp,
            tc.tile_pool(name="ebp", bufs=40) as ebpool,
            tc.tile_pool(name="ework", bufs=4) as ework,
            tc.tile_pool(name="o2", bufs=1) as o2p,
            tc.tile_pool(name="dance", bufs=3) as dancep,
            tc.tile_pool(name="outs", bufs=9) as outsp,
            tc.tile_pool(name="qkh", bufs=2, space="PSUM") as psA,
            tc.tile_pool(name="mmhalf", bufs=2, space="PSUM") as psD,
            tc.tile_pool(name="pacc", bufs=2, space="PSUM") as psB,
        ):
            ident = constp.tile([128, 128], f32)
            make_identity(nc, ident[:])
            identb = constp.tile([128, 128], bf16, name="identb")
            nc.vector.tensor_copy(identb[:], ident[:])

            wq_sb = wtp.tile([128, NC_CHUNK * 128], f8, tag="w1")
            wk_sb = wtp.tile([128, NC_CHUNK * 128], f8, tag="wk")
            wv_sb = wtp.tile([128, NC_CHUNK * 128], f8, tag="wv")
            wvc_sb = wtp.tile([128, NC_CHUNK * 128], bf16, tag="wvc")
            wo_sb = wtp.tile([128, DIM], bf16, tag="wo", name="wo_sb")
            # load order matches first use: v, k, q (proj runs v first)
            for wsb_, wdr_ in ((wv_sb, wv), (wk_sb, wk), (wq_sb, wq),
                               (wo_sb, wo), (wvc_sb, wvc)):
                nc.scalar.dma_start(out=wsb_[:], in_=wdr_[:])

            qT = [qkvp.tile([128, N], bf16, tag=f"qT{b}", name=f"qT{b}")
                  for b in range(B)]
            kT = [qkvp.tile([128, N], bf16, tag=f"kT{b}", name=f"kT{b}")
                  for b in range(B)]
            vT = [qkvp.tile([128, N], bf16, tag=f"vT{b}", name=f"vT{b}")
                  for b in range(B)]
            vn = [qkvp.tile([128, H_LOC * NT * 65], bf16, tag=f"vn{b}",
                            name=f"vn{b}") for b in range(B)]
            o2 = [o2p.tile([128, N], bf16, tag=f"o2{b}", name=f"o2{b}")
                  for b in range(B)]
            # ones columns of vn (every 65th col), written once per batch
            for b in range(B):
                nc.vector.memset(
                    vn[b][:].rearrange("p (t c) -> p t c", c=65)[:, :, 64:65],
                    1.0)

            def gen_proj(b, evac_dve=False):
                # all projections via fp8 DoubleRow (4 chunk-pairs, x32
                # weights); v gets a bf16 correction for tokens 0-127
                for irp in range(NIR // 2):
                    if b == 0 and irp == 0:
                        x8t = [xnp.tile([128, 2 * 1024], f8, tag="x8", bufs=4,
                                        name=f"x8{b}_{irp}_{g}")
                               for g in range(4)]
                        x8 = [t[:] for t in x8t]
                        for g in range(4):
                            eng = nc.scalar if g % 2 == 1 else nc.sync
                            eng.dma_start(
                                out=x8[g].rearrange("p (kt f) -> p kt f",
                                                    kt=2),
                                in_=xnT[b, g * 256:(g + 1) * 256,
                                        irp * 1024:(irp + 1) * 1024].rearrange(
                                            "(kt p) f -> p kt f", kt=2))
                    else:
                        xw = xnp.tile([128, 8 * 1024], f8, tag="x8w", bufs=2,
                                      name=f"x8w{b}_{irp}")
                        nc.sync.dma_start(
                            out=xw[:].rearrange("p (g kt f) -> p g kt f",
                                                g=4, kt=2),
                            in_=xnT[b, :, irp * 1024:(irp + 1) * 1024].rearrange(
                                "(g kt p) f -> p g kt f", g=4, kt=2))
                        x8 = [xw[:, g * 2048:(g + 1) * 2048] for g in range(4)]
                    if irp == 0:
                        xc = xnp.tile([128, NC_CHUNK * 128], bf16, tag="xc",
                                      bufs=2, name=f"xc{b}")
                        nc.sync.dma_start(
                            out=xc[:].rearrange("p (c f) -> p c f", c=NC_CHUNK),
                            in_=xnTc[b].rearrange("(c p) f -> p c f",
                                                  c=NC_CHUNK))
                    for wsb, dst in ((wv_sb, vT[b]), (wk_sb, kT[b]),
                                     (wq_sb, qT[b])):
                        wsb4 = wsb[:].rearrange("p (g kt j) -> p g kt j",
                                                g=4, kt=2)
                        for half in range(2):
                            p = psD.tile([128, 512], f32, tag="mm",
                                         name=f"pp{b}_{irp}_{half}")
                            for g in range(4):
                                nc.tensor.matmul(
                                    p[:],
                                    wsb4[:, g],
                                    x8[g].rearrange(
                                        "p (kt f) -> p kt f", kt=2)[
                                        :, :, half * 512:(half + 1) * 512],
                                    start=(g == 0), stop=(g == 3),
                                    perf_mode=mybir.MatmulPerfMode.DoubleRow)
                            sl = dst[:, irp * 1024 + half * 512:
                                     irp * 1024 + (half + 1) * 512]
                            if (evac_dve or irp == 1) and half == 0:
                                nc.vector.tensor_copy(sl, p[:])
                            else:
                                nc.scalar.copy(sl, p[:])
                        if wsb is wv_sb and irp == 0:
                            # bf16 recompute of v for tokens 0-127
                            pc = psD.tile([128, 128], f32, tag="mm",
                                          name=f"pvc{b}")
                            for c in range(NC_CHUNK):
                                nc.tensor.matmul(
                                    pc[:],
                                    wvc_sb[:, c * 128:(c + 1) * 128],
                                    xc[:, c * 128:(c + 1) * 128],
                                    start=(c == 0), stop=(c == NC_CHUNK - 1))
                            if evac_dve:
                                nc.vector.tensor_copy(vT[b][:, 0:128], pc[:])
                            else:
                                nc.scalar.copy(vT[b][:, 0:128], pc[:])
                        yield

            def gen_vt(b):
                for jt in range(NT):
                    pv = psD.tile([128, 128], bf16, tag="mm",
                                  name=f"pv{b}_{jt}")
                    nc.tensor.matmul(pv[:], vT[b][:, jt * 128:(jt + 1) * 128],
                                     identb[:], is_transpose=True)
                    for h in range(H_LOC):
                        base = (h * NT + jt) * 65
                        if b == 0 and jt < 8:
                            nc.scalar.copy(vn[b][:, base:base + 64],
                                           pv[:, h * 64:h * 64 + 64])
                        else:
                            nc.vector.tensor_copy(vn[b][:, base:base + 64],
                                                  pv[:, h * 64:h * 64 + 64])
                    if jt % 4 == 3:
                        yield

            def gen_wo(b, irp, evac_dve=True):
                # per-(half, ec) steps, half-major; stage-pipelined evac;
                # one output DMA per ec (fires after the half-1 evacuation)
                pend = None
                osb_tiles = {}

                def evac(p):
                    pw, ec, half, i = p
                    if ec not in osb_tiles:
                        osb_tiles[ec] = outsp.tile(
                            [128, 1024], bf16, tag="osb",
                            name=f"osb{b}_{ec}_{irp}")
                    osb = osb_tiles[ec]
                    dst = osb[:, half * 512:(half + 1) * 512]
                    if evac_dve is True or (evac_dve == "alt" and (ec + half) % 2 == 0):
                        nc.vector.tensor_copy(dst, pw[:])
                    else:
                        nc.scalar.copy(dst, pw[:])
                    if half == 1:
                        nc.sync.dma_start(
                            out=o[b, ec * 128:(ec + 1) * 128,
                                  irp * 1024:(irp + 1) * 1024],
                            in_=osb[:])

                i = 0
                for half in range(2):
                    for ec in range(NC_CHUNK):
                        pw = psD.tile([128, 512], f32, tag="mm",
                                      name=f"pw{b}_{ec}_{irp}_{half}")
                        nc.tensor.matmul(
                            pw[:],
                            wo_sb[:, ec * 128:(ec + 1) * 128],
                            o2[b][:, irp * 1024 + half * 512:
                                  irp * 1024 + (half + 1) * 512],
                            start=True, stop=True)
                        if pend is not None:
                            evac(pend)
                        pend = (pw, ec, half, i)
                        i += 1
                        yield
                evac(pend)

            eb_tiles = {}

            def emit_eb_ir(h, ir):
                if (h, ir) in eb_tiles:
                    return
                njt = min(4 * ir + 4, NT)
                t = ebpool.tile([128, njt * 512], bf16, tag=f"eb{ir}", bufs=2,
                                name=f"eb_{h}_{ir}")
                nc.sync.dma_start(
                    out=t[:].rearrange("p (j f) -> p j f", j=njt),
                    in_=eb[h, 0:njt * 128, ir * 512:(ir + 1) * 512].rearrange(
                        "(j p) f -> p j f", j=njt))
                eb_tiles[(h, ir)] = t

            def emit_eb_tile(h, jtp, ir):
                emit_eb_ir(h, ir)

            def gen_eb(h):
                for ir in range(NIR):
                    emit_eb_ir(h, ir)
                    yield

            def gen_attn(h, b):
                # stage-pipelined: PV (+ epilogue at ir close) for block k-1
                # is emitted after QK/exp/mul of block k, so no engine queue
                # entry ever waits long (avoids head-of-line blocking).
                pends = []

                def epilogue(po, ir):
                    rfl = dancep.tile([1, 512], bf16, tag="rfl", bufs=2,
                                      name=f"rf{h}_{b}_{ir}")
                    with nc.allow_low_precision(reason="bf16 1/denom"):
                        nc.vector.reciprocal(rfl[0:1, :], po[64:65, :])
                    rb = ework.tile([128, 512], bf16, tag="rb", bufs=2,
                                    name=f"rb{h}_{b}_{ir}")
                    for s in range(4):
                        nc.gpsimd.partition_broadcast(
                            rb[:, s * 128:(s + 1) * 128],
                            rfl[0:1, s * 128:(s + 1) * 128])
                    if h == 0:
                        nc.vector.tensor_mul(
                            o2[b][0:64, ir * 512:(ir + 1) * 512],
                            po[0:64, :], rb[0:64, :])
                    else:
                        tmp = ework.tile([64, 512], bf16, tag="hshift",
                                         bufs=2, name=f"hs{h}_{b}_{ir}")
                        nc.vector.tensor_mul(tmp[:], po[0:64, :], rb[0:64, :])
                        nc.sync.dma_start(
                            out=o2[b][64:128, ir * 512:(ir + 1) * 512],
                            in_=tmp[:])

                def stage2(p):
                    e2, jtp, jt_max, po, ir = p
                    for half in range(2):
                        jt = 2 * jtp + half
                        vbase = (h * NT + jt) * 65
                        nc.tensor.matmul(
                            po[:], vn[b][:, vbase:vbase + 65],
                            e2[:, half * 512:(half + 1) * 512],
                            start=(jt == 0), stop=(jt == jt_max - 1))
                    if 2 * jtp + 1 == jt_max - 1:
                        epilogue(po, ir)

                for ir in range(NIR):
                    jt_max = min(4 * ir + 4, NT)
                    po = psB.tile([65, 512], f32, tag="po",
                                  name=f"po{h}_{b}_{ir}")
                    for jtp in range(jt_max // 2):
                        emit_eb_ir(h, ir)
                        ebt = eb_tiles[(h, ir)][:, jtp * 1024:(jtp + 1) * 1024]
                        # last jtp of each ir is the diagonal pair: j > i for
                        # all i in the first half of the i-range, so compute
                        # only i in [256, 512) and zero-fill the rest of e2
                        diag = (jtp == jt_max // 2 - 1)
                        i0 = 256 if diag else 0
                        ps = psA.tile([128, 1024], f32, tag="qk",
                                      name=f"ps{h}_{b}_{ir}_{jtp}")
                        for half in range(2):
                            jt = 2 * jtp + half
                            nc.tensor.matmul(
                                ps[:, half * 512 + i0:(half + 1) * 512],
                                kT[b][h * 64:(h + 1) * 64,
                                      jt * 128:(jt + 1) * 128],
                                qT[b][h * 64:(h + 1) * 64,
                                      ir * 512 + i0:(ir + 1) * 512],
                                start=True, stop=True)
                        e1 = ework.tile([128, 1024], bf16, tag="e1", bufs=6,
                                        name=f"e1_{h}_{b}_{ir}_{jtp}")
                        e2 = ework.tile([128, 1024], bf16, tag="e2", bufs=6,
                                        name=f"e2_{h}_{b}_{ir}_{jtp}")
                        if diag:
                            ps3 = ps[:].rearrange("p (h f) -> p h f", h=2)
                            e13 = e1[:].rearrange("p (h f) -> p h f", h=2)
                            e23 = e2[:].rearrange("p (h f) -> p h f", h=2)
                            eb3 = ebt.rearrange("p (h f) -> p h f", h=2)
                            nc.vector.memset(e23[:, :, 0:256], 0.0)
                            nc.scalar.activation(
                                e13[:, :, 256:512], ps3[:, :, 256:512],
                                mybir.ActivationFunctionType.Exp,
                                scale=1.0 / 1024.0)
                            nc.vector.tensor_mul(e23[:, :, 256:512],
                                                 e13[:, :, 256:512],
                                                 eb3[:, :, 256:512])
                        else:
                            nc.scalar.activation(
                                e1[:], ps[:],
                                mybir.ActivationFunctionType.Exp,
                                scale=1.0 / 1024.0)
                            nc.vector.tensor_mul(e2[:], e1[:], ebt)
                        pends.append((e2, jtp, jt_max, po, ir))
                        if len(pends) > 1:
                            stage2(pends.pop(0))
                        yield
                for p_ in pends:
                    stage2(p_)
                pends.clear()

            def run(g):
                for _ in g:
                    pass

            def take(g, n):
                for _ in range(n):
                    try:
                        next(g)
                    except StopIteration:
                        return False
                return True

            def rr2(main, other, ratio=2):
                """Interleave ratio:1; stop when main exhausts (other may
                have leftovers for a later phase)."""
                while True:
                    for _ in range(ratio):
                        if not take(main, 1):
                            return
                    take(other, 1)

            def chain_steps(steps):
                for g, n in steps:
                    for _ in range(n):
                        try:
                            yield next(g)
                        except StopIteration:
                            break
                for g, _ in steps:
                    yield from g

            def zip_gens(*pairs):
                """Round-robin over (gen, per-slot count); yield per round."""
                active = [[g, c] for g, c in pairs]
                while active:
                    for it in list(active):
                        for _ in range(it[1]):
                            try:
                                next(it[0])
                            except StopIteration:
                                active.remove(it)
                                break
                        yield

            def pair(ga, gb):
                """Strict block-by-block interleave of two attention streams."""
                while True:
                    a_ok = take(ga, 1)
                    b_ok = take(gb, 1)
                    if not (a_ok or b_ok):
                        return
                    yield

            p0 = gen_proj(0)
            vt0 = gen_vt(0)
            p1 = gen_proj(1, evac_dve=True)
            v1 = gen_vt(1)
            ebz = chain_steps([(gen_eb(0), 100), (gen_eb(1), 100)])

            # intro: proj-b0 irp0 (v,k,q) + vt jt0-7, eb-h0 trickling on DMA
            for _ in range(3):
                take(p0, 1)
                take(ebz, 1)
            take(vt0, 1)

            # phase C: both b0 attention streams paired, over the rest of
            # b0-proj, all of b1-proj, and wo(b0,0) once deps complete
            paC = pair(gen_attn(0, 0), gen_attn(1, 0))
            bgC = chain_steps([(vt0, 1), (p0, 3), (vt0, 2), (p1, 6), (v1, 4)])
            for _ in range(7):
                take(paC, 1)
                take(bgC, 1)
            w00 = gen_wo(0, 0, evac_dve="alt")
            bgC2 = chain_steps([(bgC, 999), (w00, 100), (ebz, 999)])
            rr2(paC, bgC2, 1)
            run(bgC)   # safety barrier: all projections emitted
            run(w00)

            # phase E: both b1 attention streams over remaining wo work.
            paE = pair(gen_attn(0, 1), gen_attn(1, 1))
            w01 = gen_wo(0, 1, evac_dve="alt")    # deps met (phase C done)
            ww = chain_steps([(w01, 100)])
            for _ in range(14):
                take(paE, 1)
                take(ww, 1)
                if _ >= 7 and _ % 2 == 1:
                    take(ww, 1)
            run(w01)
            w10 = gen_wo(1, 0, evac_dve="alt")    # ir0-1 epilogues done
            for _ in range(4):
                take(paE, 1)
                take(w10, 4)
            run(w10)
            w11 = gen_wo(1, 1, evac_dve="alt")    # half0 needs ir2 (round ~14)
            rr2(paE, chain_steps([(w11, 8), (ebz, 999)]), 1)
            run(w11)

    nc.compile()
    return nc


def _get_nc():
    if "nc" not in _BUILT:
        _BUILT["nc"] = _build()
    return _BUILT["nc"]


def _reference_np(x, attn_bias, mask, gamma, Wq, Wkv, Wo):
    """Numpy fallback (only used if mask is not all-True)."""
    b, n, dim = x.shape
    h, dh = HEADS, DH
    l2 = np.sqrt((x.astype(np.float64) ** 2).sum(-1, keepdims=True))
    xn = x / np.maximum(l2, 1e-12) * (dim ** 0.5) * gamma
    q = (xn @ Wq.T) * (dh ** -0.5)
    kv = xn @ Wkv.T
    k, v = kv[..., :h * dh], kv[..., h * dh:]
    def to_heads(t):
        return t.reshape(b, n, h, dh).transpose(0, 2, 1, 3)
    q, k, v = to_heads(q), to_heads(k), to_heads(v)
    sim = np.einsum('bhid,bhjd->bhij', q, k) + attn_bias
    neg = -np.finfo(np.float32).max
    sim = np.where(mask[:, None, None, :], sim, neg)
    causal = np.triu(np.ones((n, n), dtype=bool), k=1)
    sim = np.where(causal, neg, sim)
    sim = sim - sim.max(-1, keepdims=True)
    e = np.exp(sim)
    attn = e / e.sum(-1, keepdims=True)
    out = np.einsum('bhij,bhjd->bhid', attn, v)
    out = out.transpose(0, 2, 1, 3).reshape(b, n, h * dh)
    return (out @ Wo.T).astype(np.float32)


def kernel(x, attn_bias, mask, gamma, Wq, Wkv, Wo, _trace=False):
    from concourse.bass_utils import run_bass_kernel_spmd

    x = np.asarray(x, dtype=np.float32)
    attn_bias = np.asarray(attn_bias, dtype=np.float32)
    mask = np.asarray(mask)
    gamma = np.asarray(gamma, dtype=np.float32)
    Wq = np.asarray(Wq, dtype=np.float32)
    Wkv = np.asarray(Wkv, dtype=np.float32)
    Wo = np.asarray(Wo, dtype=np.float32)

    if not bool(mask.all()):
        return _reference_np(x, attn_bias, mask, gamma, Wq, Wkv, Wo)

    # ---- host prep (elementwise / layout only) ----
    bf = ml_dtypes.bfloat16
    f8 = ml_dtypes.float8_e4m3
    WS = 32.0   # fp8 weight scale; q,k each carry x32 -> exp scale 1/1024,
                # v carries x32 -> folded into Wo below
    l2 = np.sqrt((x ** 2).sum(-1, keepdims=True))
    xn = x / np.maximum(l2, 1e-12) * (DIM ** 0.5) * gamma
    xnT = np.ascontiguousarray(xn.transpose(0, 2, 1))            # [B, DIM, N]
    xnT8 = xnT.astype(f8)
    xnTc = np.ascontiguousarray(xnT[:, :, :128]).astype(bf)

    tril = np.tril(np.ones((N, N), dtype=np.float32))

    def f8_layout(wT):
        """[DIM,128] -> [128, DIM] DoubleRow image:
        out[p, ((g*2 + kt)*128) + j] = wT[g*256 + kt*128 + p, j]."""
        return np.ascontiguousarray(
            wT.reshape(4, 2, 128, 128).transpose(2, 0, 1, 3).reshape(128, DIM))

    def sbuf_layout(wT):
        """[DIM,128] -> [128, DIM] SBUF image: out[p, c*128+j] = wT[c*128+p, j]."""
        return np.ascontiguousarray(
            wT.reshape(NC_CHUNK, 128, 128).transpose(1, 0, 2).reshape(128, DIM))

    in_maps = []
    for c in range(N_CORES):
        r0 = c * 128
        wq_c = f8_layout((Wq[r0:r0 + 128] * (WS * DH ** -0.5)).T).astype(f8)
        wk_c = f8_layout((Wkv[r0:r0 + 128] * WS).T).astype(f8)
        wv_c = f8_layout(
            (Wkv[HEADS * DH + r0:HEADS * DH + r0 + 128] * WS).T).astype(f8)
        wvc_c = sbuf_layout(
            (Wkv[HEADS * DH + r0:HEADS * DH + r0 + 128] * WS).T).astype(bf)
        wo_c = np.ascontiguousarray(
            (Wo[:, r0:r0 + 128] / WS).T).astype(bf)  # [128, DIM]
        bias_c = attn_bias[H_LOC * c:H_LOC * (c + 1)]            # [2, N, N]
        ebc = np.exp(bias_c) * tril                              # mask j>i
        ebc = np.ascontiguousarray(ebc.transpose(0, 2, 1)).astype(bf)  # [h,j,i]
        in_maps.append({"xnT": xnT8, "xnTc": xnTc, "wq": wq_c, "wk": wk_c,
                        "wv": wv_c, "wvc": wvc_c, "wo": wo_c, "eb": ebc})

    nc = _get_nc()
    try:
        res = run_bass_kernel_spmd(nc, in_maps,
                                   core_ids=list(range(N_CORES)),
                                   trace=_trace)
    except ModuleNotFoundError:
        # NTFF profiling hook unavailable in this environment
        res = run_bass_kernel_spmd(nc, in_maps,
                                   core_ids=list(range(N_CORES)))
    acc = res.results[0]["o"].astype(np.float32)
    for c in range(1, N_CORES):
        acc += res.results[c]["o"].astype(np.float32)
    out = np.ascontiguousarray(acc.transpose(0, 2, 1))           # [B, N, DIM]
    if _trace:
        kernel._last_results = res
    return out


# revision 87
# speedup vs baseline: 1.0215x; 1.0049x over previous
"""Trainium2 Bass kernel for dense-transformer attention block.

Reference computation (see harness):
  xn  = x / max(||x||_2, 1e-12) * sqrt(dim) * gamma          (RMSNorm-as-written)
  q   = (xn @ Wq.T) * dh^-0.5 ; k, v = split(xn @ Wkv.T)
  sim = q k^T + attn_bias ; key-pad mask ; causal mask
  out = softmax(sim) @ v @ Wo.T

Sharding: 16 heads / 8 cores = 2 heads per core (tensor parallel).
Each core computes its 2 heads' attention + its column-slice of Wo,
producing a partial output; host sums the 8 partials.

Device dataflow (per core), everything in transposed token-on-free layout:
  qT/kT/vT = W^T-stationary matmuls over xnT (bf16, N=512)
  v        = PE-transpose of vT, + ones column (softmax denominator trick)
  S^T      = kT.T-slices @ qT  (per head, causal-triangular blocks only)
  E        = exp(S^T) * exp_bias_T   (exp(bias) precomputed on host, causal-
             masked there; softmax needs no max-subtraction: |logits| < ~15)
  O^T      = v' stationary @ E  -> row 64 = denominator
  o2       = O^T[0:64] * broadcast(1/denom)   (folded into the po evacuation)
  out^T    = WoT-chunk stationary @ o2
Host prep: RMSNorm + transposes + weight folding + exp(bias) (elementwise);
all GEMMs and softmax run on device. All HBM traffic is bf16.
"""
import sys
import numpy as np

sys.path.insert(0, "/opt/trn_rl_repo")

import ml_dtypes  # noqa: E402

N_CORES = 8
B = 2
N = 2048
DIM = 1024
HEADS = 16
DH = 64
H_LOC = HEADS // N_CORES  # 2 heads per core
NT = N // 128             # 16 token tiles of 128
NIR = N // 512            # 4 i-ranges of 512
NC_CHUNK = DIM // 128     # 8 contraction chunks

_BUILT = {}


def _build():
    """Construct + compile the per-core Bass program (same for all cores)."""
    import concourse.bass as bass
    import concourse.mybir as mybir
    from concourse import bacc
    from concourse.tile import TileContext
    from concourse.masks import make_identity

    f32 = mybir.dt.float32
    bf16 = mybir.dt.bfloat16
    f8 = mybir.dt.float8e4

    nc = bacc.Bacc("TRN2", target_bir_lowering=False, debug=False,
                   num_devices=N_CORES)

    xnT = nc.dram_tensor("xnT", [B, DIM, N], f8, kind="ExternalInput").ap()
    xnTc = nc.dram_tensor("xnTc", [B, DIM, 128], bf16, kind="ExternalInput").ap()
    # q/k/v weights in fp8 DoubleRow SBUF layout (x32); wvc = bf16 x32 copy
    # of wv for the tokens 0-127 correction patch (early causal rows see few
    # keys, so v quantization error there passes straight to the output)
    wq = nc.dram_tensor("wq", [128, DIM], f8, kind="ExternalInput").ap()
    wk = nc.dram_tensor("wk", [128, DIM], f8, kind="ExternalInput").ap()
    wv = nc.dram_tensor("wv", [128, DIM], f8, kind="ExternalInput").ap()
    wvc = nc.dram_tensor("wvc", [128, DIM], bf16, kind="ExternalInput").ap()
    wo = nc.dram_tensor("wo", [128, DIM], bf16, kind="ExternalInput").ap()
    eb = nc.dram_tensor("eb", [H_LOC, N, N], bf16, kind="ExternalInput").ap()
    o = nc.dram_tensor("o", [B, DIM, N], bf16, kind="ExternalOutput").ap()

    with TileContext(nc) as tc:
        with (
            tc.tile_pool(name="const", bufs=1) as constp,
            tc.tile_pool(name="wts", bufs=1) as wtp,
            tc.tile_pool(name="xn", bufs=4) as xnp,
            tc.tile_pool(name="qkv", bufs=1) as qkvp,
            tc.tile_pool(name="ebp", bufs=40) as ebpool,
            tc.tile_pool(name="ework", bufs=4) as ework,
            tc.tile_pool(name="o2", bufs=1) as o2p,
            tc.tile_pool(name="dance", bufs=3) as dancep,
            tc.tile_pool(name="outs", bufs=9) as outsp,
            tc.tile_pool(name="qkh", bufs=2, space="PSUM") as psA,
            tc.tile_pool(name="mmhalf", bufs=2, space="PSUM") as psD,
            tc.tile_pool(name="pacc", bufs=2, space="PSUM") as psB,
        ):
            ident = constp.tile([128, 128], f32)
            make_identity(nc, ident[:])
            identb = constp.tile([128, 128], bf16, name="identb")
            nc.vector.tensor_copy(identb[:], ident[:])

            wq_sb = wtp.tile([128, NC_CHUNK * 128], f8, tag="w1")
            wk_sb = wtp.tile([128, NC_CHUNK * 128], f8, tag="wk")
            wv_sb = wtp.tile([128, NC_CHUNK * 128], f8, tag="wv")
            wvc_sb = wtp.tile([128, NC_CHUNK * 128], bf16, tag="wvc")
            wo_sb = wtp.tile([128, DIM], bf16, tag="wo", name="wo_sb")
            # load order matches first use: v, k, q (proj runs v first)
            for wsb_, wdr_ in ((wv_sb, wv), (wk_sb, wk), (wq_sb, wq),
                               (wo_sb, wo), (wvc_sb, wvc)):
                nc.scalar.dma_start(out=wsb_[:], in_=wdr_[:])

            qT = [qkvp.tile([128, N], bf16, tag=f"qT{b}", name=f"qT{b}")
                  for b in range(B)]
            kT = [qkvp.tile([128, N], bf16, tag=f"kT{b}", name=f"kT{b}")
                  for b in range(B)]
            vT = [qkvp.tile([128, N], bf16, tag=f"vT{b}", name=f"vT{b}")
                  for b in range(B)]
            vn = [qkvp.tile([128, H_LOC * NT * 65], bf16, tag=f"vn{b}",
                            name=f"vn{b}") for b in range(B)]
            o2 = [o2p.tile([128, N], bf16, tag=f"o2{b}", name=f"o2{b}")
                  for b in range(B)]
            # ones columns of vn (every 65th col), written once per batch
            for b in range(B):
                nc.vector.memset(
                    vn[b][:].rearrange("p (t c) -> p t c", c=65)[:, :, 64:65],
                    1.0)

            def gen_proj(b, evac_dve=False):
                # all projections via fp8 DoubleRow (4 chunk-pairs, x32
                # weights); v gets a bf16 correction for tokens 0-127
                for irp in range(NIR // 2):
                    if b == 0 and irp == 0:
                        x8t = [xnp.tile([128, 2 * 1024], f8, tag="x8", bufs=4,
                                        name=f"x8{b}_{irp}_{g}")
                               for g in range(4)]
                        x8 = [t[:] for t in x8t]
                        for g in range(4):
                            eng = nc.scalar if g % 2 == 1 else nc.sync
                            eng.dma_start(
                                out=x8[g].rearrange("p (kt f) -> p kt f",
                                                    kt=2),
                                in_=xnT[b, g * 256:(g + 1) * 256,
                                        irp * 1024:(irp + 1) * 1024].rearrange(
                                            "(kt p) f -> p kt f", kt=2))
                    else:
                        xw = xnp.tile([128, 8 * 1024], f8, tag="x8w", bufs=2,
                                      name=f"x8w{b}_{irp}")
                        nc.sync.dma_start(
                            out=xw[:].rearrange("p (g kt f) -> p g kt f",
                                                g=4, kt=2),
                            in_=xnT[b, :, irp * 1024:(irp + 1) * 1024].rearrange(
                                "(g kt p) f -> p g kt f", g=4, kt=2))
                        x8 = [xw[:, g * 2048:(g + 1) * 2048] for g in range(4)]
                    if irp == 0:
                        xc = xnp.tile([128, NC_CHUNK * 128], bf16, tag="xc",
                                      bufs=2, name=f"xc{b}")
                        nc.sync.dma_start(
                            out=xc[:].rearrange("p (c f) -> p c f", c=NC_CHUNK),
                            in_=xnTc[b].rearrange("(c p) f -> p c f",
                                                  c=NC_CHUNK))
                    for wsb, dst in ((wv_sb, vT[b]), (wk_sb, kT[b]),
                                     (wq_sb, qT[b])):
                        wsb4 = wsb[:].rearrange("p (g kt j) -> p g kt j",
                                                g=4, kt=2)
                        for half in range(2):
                            p = psD.tile([128, 512], f32, tag="mm",
                                         name=f"pp{b}_{irp}_{half}")
                            for g in range(4):
                                nc.tensor.matmul(
                                    p[:],
                                    wsb4[:, g],
                                    x8[g].rearrange(
                                        "p (kt f) -> p kt f", kt=2)[
                                        :, :, half * 512:(half + 1) * 512],
                                    start=(g == 0), stop=(g == 3),
                                    perf_mode=mybir.MatmulPerfMode.DoubleRow)
                            sl = dst[:, irp * 1024 + half * 512:
                                     irp * 1024 + (half + 1) * 512]
                            if (evac_dve or irp == 1) and half == 0:
                                nc.vector.tensor_copy(sl, p[:])
                            else:
                                nc.scalar.copy(sl, p[:])
                        if wsb is wv_sb and irp == 0:
                            # bf16 recompute of v for tokens 0-127
                            pc = psD.tile([128, 128], f32, tag="mm",
                                          name=f"pvc{b}")
                            for c in range(NC_CHUNK):
                                nc.tensor.matmul(
                                    pc[:],
                                    wvc_sb[:, c * 128:(c + 1) * 128],
                                    xc[:, c * 128:(c + 1) * 128],
                                    start=(c == 0), stop=(c == NC_CHUNK - 1))
                            if evac_dve:
                                nc.vector.tensor_copy(vT[b][:, 0:128], pc[:])
                            else:
                                nc.scalar.copy(vT[b][:, 0:128], pc[:])
                        yield

            def gen_vt(b):
                for jt in range(NT):
                    pv = psD.tile([128, 128], bf16, tag="mm",
                                  name=f"pv{b}_{jt}")
                    nc.tensor.matmul(pv[:], vT[b][:, jt * 128:(jt + 1) * 128],
                                     identb[:], is_transpose=True)
                    for h in range(H_LOC):
                        base = (h * NT + jt) * 65
                        if b == 0 and jt < 8:
                            nc.scalar.copy(vn[b][:, base:base + 64],
                                           pv[:, h * 64:h * 64 + 64])
                        else:
                            nc.vector.tensor_copy(vn[b][:, base:base + 64],
                                                  pv[:, h * 64:h * 64 + 64])
                    if jt % 4 == 3:
                        yield

            def gen_wo(b, irp, evac_dve=True):
                # per-(half, ec) steps, half-major; stage-pipelined evac;
                # one output DMA per ec (fires after the half-1 evacuation)
                pend = None
                osb_tiles = {}

                def evac(p):
                    pw, ec, half, i = p
                    if ec not in osb_tiles:
                        osb_tiles[ec] = outsp.tile(
                            [128, 1024], bf16, tag="osb",
                            name=f"osb{b}_{ec}_{irp}")
                    osb = osb_tiles[ec]
                    dst = osb[:, half * 512:(half + 1) * 512]
                    if evac_dve is True or (evac_dve == "alt" and (ec + half) % 2 == 0):
                        nc.vector.tensor_copy(dst, pw[:])
                    else:
                        nc.scalar.copy(dst, pw[:])
                    if half == 1:
                        nc.sync.dma_start(
                            out=o[b, ec * 128:(ec + 1) * 128,
                                  irp * 1024:(irp + 1) * 1024],
                            in_=osb[:])

                i = 0
                for half in range(2):
                    for ec in range(NC_CHUNK):
                        pw = psD.tile([128, 512], f32, tag="mm",
                                      name=f"pw{b}_{ec}_{irp}_{half}")
                        nc.tensor.matmul(
                            pw[:],
                            wo_sb[:, ec * 128:(ec + 1) * 128],
                            o2[b][:, irp * 1024 + half * 512:
                                  irp * 1024 + (half + 1) * 512],
                            start=True, stop=True)
                        if pend is not None:
                            evac(pend)
                        pend = (pw, ec, half, i)
                        i += 1
                        yield
                evac(pend)

            eb_tiles = {}

            def emit_eb_ir(h, ir):
                if (h, ir) in eb_tiles:
                    return
                njt = min(4 * ir + 4, NT)
                t = ebpool.tile([128, njt * 512], bf16, tag=f"eb{ir}", bufs=2,
                                name=f"eb_{h}_{ir}")
                nc.sync.dma_start(
                    out=t[:].rearrange("p (j f) -> p j f", j=njt),
                    in_=eb[h, 0:njt * 128, ir * 512:(ir + 1) * 512].rearrange(
                        "(j p) f -> p j f", j=njt))
                eb_tiles[(h, ir)] = t

            def emit_eb_tile(h, jtp, ir):
                emit_eb_ir(h, ir)

            def gen_eb(h):
                for ir in range(NIR):
                    emit_eb_ir(h, ir)
                    yield

            def gen_attn(h, b):
                # stage-pipelined: PV (+ epilogue at ir close) for block k-1
                # is emitted after QK/exp/mul of block k, so no engine queue
                # entry ever waits long (avoids head-of-line blocking).
                pends = []

                def epilogue(po, ir):
                    rfl = dancep.tile([1, 512], bf16, tag="rfl", bufs=2,
                                      name=f"rf{h}_{b}_{ir}")
                    with nc.allow_low_precision(reason="bf16 1/denom"):
                        nc.vector.reciprocal(rfl[0:1, :], po[64:65, :])
                    rb = ework.tile([128, 512], bf16, tag="rb", bufs=2,
                                    name=f"rb{h}_{b}_{ir}")
                    for s in range(4):
                        nc.gpsimd.partition_broadcast(
                            rb[:, s * 128:(s + 1) * 128],
                            rfl[0:1, s * 128:(s + 1) * 128])
                    if h == 0:
                        nc.vector.tensor_mul(
                            o2[b][0:64, ir * 512:(ir + 1) * 512],
                            po[0:64, :], rb[0:64, :])
                    else:
                        tmp = ework.tile([64, 512], bf16, tag="hshift",
                                         bufs=2, name=f"hs{h}_{b}_{ir}")
                        nc.vector.tensor_mul(tmp[:], po[0:64, :], rb[0:64, :])
                        nc.sync.dma_start(
                            out=o2[b][64:128, ir * 512:(ir + 1) * 512],
                            in_=tmp[:])

                def stage2(p):
                    e2, jtp, jt_max, po, ir = p
                    for half in range(2):
                        jt = 2 * jtp + half
                        vbase = (h * NT + jt) * 65
                        nc.tensor.matmul(
                            po[:], vn[b][:, vbase:vbase + 65],
                            e2[:, half * 512:(half + 1) * 512],
                            start=(jt == 0), stop=(jt == jt_max - 1))
                    if 2 * jtp + 1 == jt_max - 1:
                        epilogue(po, ir)

                for ir in range(NIR):
                    jt_max = min(4 * ir + 4, NT)
                    po = psB.tile([65, 512], f32, tag="po",
                                  name=f"po{h}_{b}_{ir}")
                    for jtp in range(jt_max // 2):
                        emit_eb_ir(h, ir)
                        ebt = eb_tiles[(h, ir)][:, jtp * 1024:(jtp + 1) * 1024]
                        # last jtp of each ir is the diagonal pair: j > i for
                        # all i in the first half of the i-range, so compute
                        # only i in [256, 512) and zero-fill the rest of e2
                        diag = (jtp == jt_max // 2 - 1)
                        i0 = 256 if diag else 0
                        ps = psA.tile([128, 1024], f32, tag="qk",
                                      name=f"ps{h}_{b}_{ir}_{jtp}")
                        for half in range(2):
                            jt = 2 * jtp + half
                            nc.tensor.matmul(
                                ps[:, half * 512 + i0:(half + 1) * 512],
                                kT[b][h * 64:(h + 1) * 64,
                                      jt * 128:(jt + 1) * 128],
                                qT[b][h * 64:(h + 1) * 64,
                                      ir * 512 + i0:(ir + 1) * 512],
                                start=True, stop=True)
                        e1 = ework.tile([128, 1024], bf16, tag="e1", bufs=6,
                                        name=f"e1_{h}_{b}_{ir}_{jtp}")
                        e2 = ework.tile([128, 1024], bf16, tag="e2", bufs=6,
                                        name=f"e2_{h}_{b}_{ir}_{jtp}")
                        if diag:
                            ps3 = ps[:].rearrange("p (h f) -> p h f", h=2)
                            e13 = e1[:].rearrange("p (h f) -> p h f", h=2)
                            e23 = e2[:].rearrange("p (h f) -> p h f", h=2)
                            eb3 = ebt.rearrange("p (h f) -> p h f", h=2)
                            nc.gpsimd.memset(e23[:, :, 0:256], 0.0)
                            nc.scalar.activation(
                                e13[:, :, 256:512], ps3[:, :, 256:512],
                                mybir.ActivationFunctionType.Exp,
                                scale=1.0 / 1024.0)
                            nc.vector.tensor_mul(e23[:, :, 256:512],
                                                 e13[:, :, 256:512],
                                                 eb3[:, :, 256:512])
                        else:
                            nc.scalar.activation(
                                e1[:], ps[:],
                                mybir.ActivationFunctionType.Exp,
                                scale=1.0 / 1024.0)
                            nc.vector.tensor_mul(e2[:], e1[:], ebt)
                        pends.append((e2, jtp, jt_max, po, ir))
                        if len(pends) > 1:
                            stage2(pends.pop(0))
                        yield
                for p_ in pends:
                    stage2(p_)
                pends.clear()

            def run(g):
                for _ in g:
                    pass

            def take(g, n):
                for _ in range(n):
                    try:
                        next(g)
                    except StopIteration:
                        return False
                return True

            def rr2(main, other, ratio=2):
                """Interleave ratio:1; stop when main exhausts (other may
                have leftovers for a later phase)."""
                while True:
                    for _ in range(ratio):
                        if not take(main, 1):
                            return
                    take(other, 1)

            def chain_steps(steps):
                for g, n in steps:
                    for _ in range(n):
                        try:
                            yield next(g)
                        except StopIteration:
                            break
                for g, _ in steps:
                    yield from g

            def zip_gens(*pairs):
                """Round-robin over (gen, per-slot count); yield per round."""
                active = [[g, c] for g, c in pairs]
                while active:
                    for it in list(active):
                        for _ in range(it[1]):
                            try:
                                next(it[0])
                            except StopIteration:
                                active.remove(it)
                                break
                        yield

            def pair(ga, gb):
                """Strict block-by-block interleave of two attention streams."""
                while True:
                    a_ok = take(ga, 1)
                    b_ok = take(gb, 1)
                    if not (a_ok or b_ok):
                        return
                    yield

            p0 = gen_proj(0)
            vt0 = gen_vt(0)
            p1 = gen_proj(1, evac_dve=True)
            v1 = gen_vt(1)
            ebz = chain_steps([(gen_eb(0), 100), (gen_eb(1), 100)])

            # intro: proj-b0 irp0 (v,k,q) + vt jt0-7, eb-h0 trickling on DMA
            for _ in range(3):
                take(p0, 1)
                take(ebz, 1)
            take(vt0, 1)

            # phase C: both b0 attention streams paired, over the rest of
            # b0-proj, all of b1-proj, and wo(b0,0) once deps complete
            paC = pair(gen_attn(0, 0), gen_attn(1, 0))
            bgC = chain_steps([(vt0, 1), (p0, 3), (vt0, 2), (p1, 6), (v1, 4)])
            for _ in range(7):
                take(paC, 1)
                take(bgC, 1)
            w00 = gen_wo(0, 0, evac_dve="alt")
            bgC2 = chain_steps([(bgC, 999), (w00, 100), (ebz, 999)])
            rr2(paC, bgC2, 1)
            run(bgC)   # safety barrier: all projections emitted
            run(w00)

            # phase E: both b1 attention streams over remaining wo work.
            paE = pair(gen_attn(0, 1), gen_attn(1, 1))
            w01 = gen_wo(0, 1, evac_dve="alt")    # deps met (phase C done)
            ww = chain_steps([(w01, 100)])
            for _ in range(14):
                take(paE, 1)
                take(ww, 1)
                if _ >= 7 and _ % 2 == 1:
                    take(ww, 1)
            run(w01)
            w10 = gen_wo(1, 0, evac_dve="alt")    # ir0-1 epilogues done
            for _ in range(4):
                take(paE, 1)
                take(w10, 4)
            run(w10)
            w11 = gen_wo(1, 1, evac_dve="alt")    # half0 needs ir2 (round ~14)
            rr2(paE, chain_steps([(w11, 8), (ebz, 999)]), 1)
            run(w11)

    nc.compile()
    return nc


def _get_nc():
    if "nc" not in _BUILT:
        _BUILT["nc"] = _build()
    return _BUILT["nc"]


def _reference_np(x, attn_bias, mask, gamma, Wq, Wkv, Wo):
    """Numpy fallback (only used if mask is not all-True)."""
    b, n, dim = x.shape
    h, dh = HEADS, DH
    l2 = np.sqrt((x.astype(np.float64) ** 2).sum(-1, keepdims=True))
    xn = x / np.maximum(l2, 1e-12) * (dim ** 0.5) * gamma
    q = (xn @ Wq.T) * (dh ** -0.5)
    kv = xn @ Wkv.T
    k, v = kv[..., :h * dh], kv[..., h * dh:]
    def to_heads(t):
        return t.reshape(b, n, h, dh).transpose(0, 2, 1, 3)
    q, k, v = to_heads(q), to_heads(k), to_heads(v)
    sim = np.einsum('bhid,bhjd->bhij', q, k) + attn_bias
    neg = -np.finfo(np.float32).max
    sim = np.where(mask[:, None, None, :], sim, neg)
    causal = np.triu(np.ones((n, n), dtype=bool), k=1)
    sim = np.where(causal, neg, sim)
    sim = sim - sim.max(-1, keepdims=True)
    e = np.exp(sim)
    attn = e / e.sum(-1, keepdims=True)
    out = np.einsum('bhij,bhjd->bhid', attn, v)
    out = out.transpose(0, 2, 1, 3).reshape(b, n, h * dh)
    return (out @ Wo.T).astype(np.float32)


def kernel(x, attn_bias, mask, gamma, Wq, Wkv, Wo, _trace=False):
    from concourse.bass_utils import run_bass_kernel_spmd

    x = np.asarray(x, dtype=np.float32)
    attn_bias = np.asarray(attn_bias, dtype=np.float32)
    mask = np.asarray(mask)
    gamma = np.asarray(gamma, dtype=np.float32)
    Wq = np.asarray(Wq, dtype=np.float32)
    Wkv = np.asarray(Wkv, dtype=np.float32)
    Wo = np.asarray(Wo, dtype=np.float32)

    if not bool(mask.all()):
        return _reference_np(x, attn_bias, mask, gamma, Wq, Wkv, Wo)

    # ---- host prep (elementwise / layout only) ----
    bf = ml_dtypes.bfloat16
    f8 = ml_dtypes.float8_e4m3
    WS = 32.0   # fp8 weight scale; q,k each carry x32 -> exp scale 1/1024,
                # v carries x32 -> folded into Wo below
    l2 = np.sqrt((x ** 2).sum(-1, keepdims=True))
    xn = x / np.maximum(l2, 1e-12) * (DIM ** 0.5) * gamma
    xnT = np.ascontiguousarray(xn.transpose(0, 2, 1))            # [B, DIM, N]
    xnT8 = xnT.astype(f8)
    xnTc = np.ascontiguousarray(xnT[:, :, :128]).astype(bf)

    tril = np.tril(np.ones((N, N), dtype=np.float32))

    def f8_layout(wT):
        """[DIM,128] -> [128, DIM] DoubleRow image:
        out[p, ((g*2 + kt)*128) + j] = wT[g*256 + kt*128 + p, j]."""
        return np.ascontiguousarray(
            wT.reshape(4, 2, 128, 128).transpose(2, 0, 1, 3).reshape(128, DIM))

    def sbuf_layout(wT):
        """[DIM,128] -> [128, DIM] SBUF image: out[p, c*128+j] = wT[c*128+p, j]."""
        return np.ascontiguousarray(
            wT.reshape(NC_CHUNK, 128, 128).transpose(1, 0, 2).reshape(128, DIM))

    in_maps = []
    for c in range(N_CORES):
        r0 = c * 128
        wq_c = f8_layout((Wq[r0:r0 + 128] * (WS * DH ** -0.5)).T).astype(f8)
        wk_c = f8_layout((Wkv[r0:r0 + 128] * WS).T).astype(f8)
        wv_c = f8_layout(
            (Wkv[HEADS * DH + r0:HEADS * DH + r0 + 128] * WS).T).astype(f8)
        wvc_c = sbuf_layout(
            (Wkv[HEADS * DH + r0:HEADS * DH + r0 + 128] * WS).T).astype(bf)
        wo_c = np.ascontiguousarray(
            (Wo[:, r0:r0 + 128] / WS).T).astype(bf)  # [128, DIM]
        bias_c = attn_bias[H_LOC * c:H_LOC * (c + 1)]            # [2, N, N]
        ebc = np.exp(bias_c) * tril                              # mask j>i
        ebc = np.ascontiguousarray(ebc.transpose(0, 2, 1)).astype(bf)  # [h,j,i]
        in_maps.append({"xnT": xnT8, "xnTc": xnTc, "wq": wq_c, "wk": wk_c,
                        "wv": wv_c, "wvc": wvc_c, "wo": wo_c, "eb": ebc})

    nc = _get_nc()
    try:
        res = run_bass_kernel_spmd(nc, in_maps,
                                   core_ids=list(range(N_CORES)),
                                   trace=_trace)
    except ModuleNotFoundError:
        # NTFF profiling hook unavailable in this environment
        res = run_bass_kernel_spmd(nc, in_maps,
                                   core_ids=list(range(N_CORES)))
    acc = res.results[0]["o"].astype(np.float32)
    for c in range(1, N_CORES):
        acc += res.results[c]["o"].astype(np.float32)
    out = np.ascontiguousarray(acc.transpose(0, 2, 1))           # [B, N, DIM]
    if _trace:
        kernel._last_results = res
    return out


# revision 88
# speedup vs baseline: 1.0541x; 1.0320x over previous
"""Trainium2 Bass kernel for dense-transformer attention block.

Reference computation (see harness):
  xn  = x / max(||x||_2, 1e-12) * sqrt(dim) * gamma          (RMSNorm-as-written)
  q   = (xn @ Wq.T) * dh^-0.5 ; k, v = split(xn @ Wkv.T)
  sim = q k^T + attn_bias ; key-pad mask ; causal mask
  out = softmax(sim) @ v @ Wo.T

Sharding: 16 heads / 8 cores = 2 heads per core (tensor parallel).
Each core computes its 2 heads' attention + its column-slice of Wo,
producing a partial output; host sums the 8 partials.

Device dataflow (per core), everything in transposed token-on-free layout:
  qT/kT/vT = W^T-stationary matmuls over xnT (bf16, N=512)
  v        = PE-transpose of vT, + ones column (softmax denominator trick)
  S^T      = kT.T-slices @ qT  (per head, causal-triangular blocks only)
  E        = exp(S^T) * exp_bias_T   (exp(bias) precomputed on host, causal-
             masked there; softmax needs no max-subtraction: |logits| < ~15)
  O^T      = v' stationary @ E  -> row 64 = denominator
  o2       = O^T[0:64] * broadcast(1/denom)   (folded into the po evacuation)
  out^T    = WoT-chunk stationary @ o2
Host prep: RMSNorm + transposes + weight folding + exp(bias) (elementwise);
all GEMMs and softmax run on device. All HBM traffic is bf16.
"""
import sys
import numpy as np

sys.path.insert(0, "/opt/trn_rl_repo")

import ml_dtypes  # noqa: E402

N_CORES = 8
B = 2
N = 2048
DIM = 1024
HEADS = 16
DH = 64
H_LOC = HEADS // N_CORES  # 2 heads per core
NT = N // 128             # 16 token tiles of 128
NIR = N // 512            # 4 i-ranges of 512
NC_CHUNK = DIM // 128     # 8 contraction chunks

_BUILT = {}


def _build():
    """Construct + compile the per-core Bass program (same for all cores)."""
    import concourse.bass as bass
    import concourse.mybir as mybir
    from concourse import bacc
    from concourse.tile import TileContext
    from concourse.masks import make_identity

    f32 = mybir.dt.float32
    bf16 = mybir.dt.bfloat16
    f8 = mybir.dt.float8e4

    nc = bacc.Bacc("TRN2", target_bir_lowering=False, debug=False,
                   num_devices=N_CORES)

    xnT = nc.dram_tensor("xnT", [B, DIM, N], f8, kind="ExternalInput").ap()
    xnTc = nc.dram_tensor("xnTc", [B, DIM, 128], bf16, kind="ExternalInput").ap()
    # q/k/v weights in fp8 DoubleRow SBUF layout (x32); wvc = bf16 x32 copy
    # of wv for the tokens 0-127 correction patch (early causal rows see few
    # keys, so v quantization error there passes straight to the output)
    wq = nc.dram_tensor("wq", [128, DIM], f8, kind="ExternalInput").ap()
    wk = nc.dram_tensor("wk", [128, DIM], f8, kind="ExternalInput").ap()
    wv = nc.dram_tensor("wv", [128, DIM], f8, kind="ExternalInput").ap()
    wvc = nc.dram_tensor("wvc", [128, DIM], bf16, kind="ExternalInput").ap()
    wo = nc.dram_tensor("wo", [128, DIM], bf16, kind="ExternalInput").ap()
    eb = nc.dram_tensor("eb", [H_LOC, N, N], bf16, kind="ExternalInput").ap()
    o = nc.dram_tensor("o", [B, DIM, N], bf16, kind="ExternalOutput").ap()

    with TileContext(nc) as tc:
        with (
            tc.tile_pool(name="const", bufs=1) as constp,
            tc.tile_pool(name="wts", bufs=1) as wtp,
            tc.tile_pool(name="xn", bufs=4) as xnp,
            tc.tile_pool(name="qkv", bufs=1) as qkvp,
            tc.tile_pool(name="ebp", bufs=40) as ebpool,
            tc.tile_pool(name="ework", bufs=4) as ework,
            tc.tile_pool(name="o2", bufs=1) as o2p,
            tc.tile_pool(name="dance", bufs=3) as dancep,
            tc.tile_pool(name="outs", bufs=9) as outsp,
            tc.tile_pool(name="qkh", bufs=2, space="PSUM") as psA,
            tc.tile_pool(name="mmhalf", bufs=2, space="PSUM") as psD,
            tc.tile_pool(name="pacc", bufs=2, space="PSUM") as psB,
        ):
            ident = constp.tile([128, 128], f32)
            make_identity(nc, ident[:])
            identb = constp.tile([128, 128], bf16, name="identb")
            nc.vector.tensor_copy(identb[:], ident[:])

            wq_sb = wtp.tile([128, NC_CHUNK * 128], f8, tag="w1")
            wk_sb = wtp.tile([128, NC_CHUNK * 128], f8, tag="wk")
            wv_sb = wtp.tile([128, NC_CHUNK * 128], f8, tag="wv")
            wvc_sb = wtp.tile([128, NC_CHUNK * 128], bf16, tag="wvc")
            wo_sb = wtp.tile([128, DIM], bf16, tag="wo", name="wo_sb")
            # load order matches first use: v, k, q (proj runs v first)
            for wsb_, wdr_ in ((wv_sb, wv), (wk_sb, wk), (wq_sb, wq),
                               (wo_sb, wo), (wvc_sb, wvc)):
                nc.scalar.dma_start(out=wsb_[:], in_=wdr_[:])

            qT = [qkvp.tile([128, N], bf16, tag=f"qT{b}", name=f"qT{b}")
                  for b in range(B)]
            kT = [qkvp.tile([128, N], bf16, tag=f"kT{b}", name=f"kT{b}")
                  for b in range(B)]
            vT = [qkvp.tile([128, N], bf16, tag=f"vT{b}", name=f"vT{b}")
                  for b in range(B)]
            vn = [qkvp.tile([128, H_LOC * NT * 65], bf16, tag=f"vn{b}",
                            name=f"vn{b}") for b in range(B)]
            o2 = [o2p.tile([128, N], bf16, tag=f"o2{b}", name=f"o2{b}")
                  for b in range(B)]
            # ones columns of vn (every 65th col), written once per batch
            for b in range(B):
                nc.vector.memset(
                    vn[b][:].rearrange("p (t c) -> p t c", c=65)[:, :, 64:65],
                    1.0)

            def gen_proj(b, evac_dve=False):
                # all projections via fp8 DoubleRow (4 chunk-pairs, x32
                # weights); v gets a bf16 correction for tokens 0-127
                for irp in range(NIR // 2):
                    if b == 0 and irp == 0:
                        x8t = [xnp.tile([128, 2 * 1024], f8, tag="x8", bufs=4,
                                        name=f"x8{b}_{irp}_{g}")
                               for g in range(4)]
                        x8 = [t[:] for t in x8t]
                        for g in range(4):
                            nc.sync.dma_start(
                                out=x8[g].rearrange("p (kt f) -> p kt f",
                                                    kt=2),
                                in_=xnT[b, g * 256:(g + 1) * 256,
                                        irp * 1024:(irp + 1) * 1024].rearrange(
                                            "(kt p) f -> p kt f", kt=2))
                    else:
                        xw = xnp.tile([128, 8 * 1024], f8, tag="x8w", bufs=2,
                                      name=f"x8w{b}_{irp}")
                        nc.sync.dma_start(
                            out=xw[:].rearrange("p (g kt f) -> p g kt f",
                                                g=4, kt=2),
                            in_=xnT[b, :, irp * 1024:(irp + 1) * 1024].rearrange(
                                "(g kt p) f -> p g kt f", g=4, kt=2))
                        x8 = [xw[:, g * 2048:(g + 1) * 2048] for g in range(4)]
                    if irp == 0:
                        xc = xnp.tile([128, NC_CHUNK * 128], bf16, tag="xc",
                                      bufs=2, name=f"xc{b}")
                        nc.sync.dma_start(
                            out=xc[:].rearrange("p (c f) -> p c f", c=NC_CHUNK),
                            in_=xnTc[b].rearrange("(c p) f -> p c f",
                                                  c=NC_CHUNK))
                    for wsb, dst in ((wv_sb, vT[b]), (wk_sb, kT[b]),
                                     (wq_sb, qT[b])):
                        wsb4 = wsb[:].rearrange("p (g kt j) -> p g kt j",
                                                g=4, kt=2)
                        for half in range(2):
                            p = psD.tile([128, 512], f32, tag="mm",
                                         name=f"pp{b}_{irp}_{half}")
                            for g in range(4):
                                nc.tensor.matmul(
                                    p[:],
                                    wsb4[:, g],
                                    x8[g].rearrange(
                                        "p (kt f) -> p kt f", kt=2)[
                                        :, :, half * 512:(half + 1) * 512],
                                    start=(g == 0), stop=(g == 3),
                                    perf_mode=mybir.MatmulPerfMode.DoubleRow)
                            sl = dst[:, irp * 1024 + half * 512:
                                     irp * 1024 + (half + 1) * 512]
                            if (evac_dve or irp == 1) and half == 0:
                                nc.vector.tensor_copy(sl, p[:])
                            else:
                                nc.scalar.copy(sl, p[:])
                        if wsb is wv_sb and irp == 0:
                            # bf16 recompute of v for tokens 0-127
                            pc = psD.tile([128, 128], f32, tag="mm",
                                          name=f"pvc{b}")
                            for c in range(NC_CHUNK):
                                nc.tensor.matmul(
                                    pc[:],
                                    wvc_sb[:, c * 128:(c + 1) * 128],
                                    xc[:, c * 128:(c + 1) * 128],
                                    start=(c == 0), stop=(c == NC_CHUNK - 1))
                            if evac_dve:
                                nc.vector.tensor_copy(vT[b][:, 0:128], pc[:])
                            else:
                                nc.scalar.copy(vT[b][:, 0:128], pc[:])
                        yield

            def gen_vt(b):
                for jt in range(NT):
                    pv = psD.tile([128, 128], bf16, tag="mm",
                                  name=f"pv{b}_{jt}")
                    nc.tensor.matmul(pv[:], vT[b][:, jt * 128:(jt + 1) * 128],
                                     identb[:], is_transpose=True)
                    for h in range(H_LOC):
                        base = (h * NT + jt) * 65
                        if b == 0 and jt < 8:
                            nc.scalar.copy(vn[b][:, base:base + 64],
                                           pv[:, h * 64:h * 64 + 64])
                        else:
                            nc.vector.tensor_copy(vn[b][:, base:base + 64],
                                                  pv[:, h * 64:h * 64 + 64])
                    if jt % 4 == 3:
                        yield

            def gen_wo(b, irp, evac_dve=True):
                # per-(half, ec) steps, half-major; stage-pipelined evac;
                # one output DMA per ec (fires after the half-1 evacuation)
                pend = None
                osb_tiles = {}

                def evac(p):
                    pw, ec, half, i = p
                    if ec not in osb_tiles:
                        osb_tiles[ec] = outsp.tile(
                            [128, 1024], bf16, tag="osb",
                            name=f"osb{b}_{ec}_{irp}")
                    osb = osb_tiles[ec]
                    dst = osb[:, half * 512:(half + 1) * 512]
                    if evac_dve is True or (evac_dve == "alt" and (ec + half) % 2 == 0):
                        nc.vector.tensor_copy(dst, pw[:])
                    else:
                        nc.scalar.copy(dst, pw[:])
                    if half == 1:
                        nc.sync.dma_start(
                            out=o[b, ec * 128:(ec + 1) * 128,
                                  irp * 1024:(irp + 1) * 1024],
                            in_=osb[:])

                i = 0
                for half in range(2):
                    for ec in range(NC_CHUNK):
                        pw = psD.tile([128, 512], f32, tag="mm",
                                      name=f"pw{b}_{ec}_{irp}_{half}")
                        nc.tensor.matmul(
                            pw[:],
                            wo_sb[:, ec * 128:(ec + 1) * 128],
                            o2[b][:, irp * 1024 + half * 512:
                                  irp * 1024 + (half + 1) * 512],
                            start=True, stop=True)
                        if pend is not None:
                            evac(pend)
                        pend = (pw, ec, half, i)
                        i += 1
                        yield
                evac(pend)

            eb_tiles = {}

            def emit_eb_ir(h, ir):
                if (h, ir) in eb_tiles:
                    return
                njt = min(4 * ir + 4, NT)
                t = ebpool.tile([128, njt * 512], bf16, tag=f"eb{ir}", bufs=2,
                                name=f"eb_{h}_{ir}")
                nc.sync.dma_start(
                    out=t[:].rearrange("p (j f) -> p j f", j=njt),
                    in_=eb[h, 0:njt * 128, ir * 512:(ir + 1) * 512].rearrange(
                        "(j p) f -> p j f", j=njt))
                eb_tiles[(h, ir)] = t

            def emit_eb_tile(h, jtp, ir):
                emit_eb_ir(h, ir)

            def gen_eb(h):
                for ir in range(NIR):
                    emit_eb_ir(h, ir)
                    yield

            def gen_attn(h, b):
                # stage-pipelined: PV (+ epilogue at ir close) for block k-1
                # is emitted after QK/exp/mul of block k, so no engine queue
                # entry ever waits long (avoids head-of-line blocking).
                pends = []

                def epilogue(po, ir):
                    rfl = dancep.tile([1, 512], bf16, tag="rfl", bufs=2,
                                      name=f"rf{h}_{b}_{ir}")
                    with nc.allow_low_precision(reason="bf16 1/denom"):
                        nc.vector.reciprocal(rfl[0:1, :], po[64:65, :])
                    rb = ework.tile([128, 512], bf16, tag="rb", bufs=2,
                                    name=f"rb{h}_{b}_{ir}")
                    for s in range(4):
                        nc.gpsimd.partition_broadcast(
                            rb[:, s * 128:(s + 1) * 128],
                            rfl[0:1, s * 128:(s + 1) * 128])
                    if h == 0:
                        nc.vector.tensor_mul(
                            o2[b][0:64, ir * 512:(ir + 1) * 512],
                            po[0:64, :], rb[0:64, :])
                    else:
                        tmp = ework.tile([64, 512], bf16, tag="hshift",
                                         bufs=2, name=f"hs{h}_{b}_{ir}")
                        nc.vector.tensor_mul(tmp[:], po[0:64, :], rb[0:64, :])
                        nc.sync.dma_start(
                            out=o2[b][64:128, ir * 512:(ir + 1) * 512],
                            in_=tmp[:])

                def stage2(p):
                    e2, jtp, jt_max, po, ir = p
                    for half in range(2):
                        jt = 2 * jtp + half
                        vbase = (h * NT + jt) * 65
                        nc.tensor.matmul(
                            po[:], vn[b][:, vbase:vbase + 65],
                            e2[:, half * 512:(half + 1) * 512],
                            start=(jt == 0), stop=(jt == jt_max - 1))
                    if 2 * jtp + 1 == jt_max - 1:
                        epilogue(po, ir)

                for ir in range(NIR):
                    jt_max = min(4 * ir + 4, NT)
                    po = psB.tile([65, 512], f32, tag="po",
                                  name=f"po{h}_{b}_{ir}")
                    for jtp in range(jt_max // 2):
                        emit_eb_ir(h, ir)
                        ebt = eb_tiles[(h, ir)][:, jtp * 1024:(jtp + 1) * 1024]
                        # last jtp of each ir is the diagonal pair: j > i for
                        # all i in the first half of the i-range, so compute
                        # only i in [256, 512) and zero-fill the rest of e2
                        diag = (jtp == jt_max // 2 - 1)
                        i0 = 256 if diag else 0
                        ps = psA.tile([128, 1024], f32, tag="qk",
                                      name=f"ps{h}_{b}_{ir}_{jtp}")
                        for half in range(2):
                            jt = 2 * jtp + half
                            nc.tensor.matmul(
                                ps[:, half * 512 + i0:(half + 1) * 512],
                                kT[b][h * 64:(h + 1) * 64,
                                      jt * 128:(jt + 1) * 128],
                                qT[b][h * 64:(h + 1) * 64,
                                      ir * 512 + i0:(ir + 1) * 512],
                                start=True, stop=True)
                        e1 = ework.tile([128, 1024], bf16, tag="e1", bufs=6,
                                        name=f"e1_{h}_{b}_{ir}_{jtp}")
                        e2 = ework.tile([128, 1024], bf16, tag="e2", bufs=6,
                                        name=f"e2_{h}_{b}_{ir}_{jtp}")
                        if diag:
                            ps3 = ps[:].rearrange("p (h f) -> p h f", h=2)
                            e13 = e1[:].rearrange("p (h f) -> p h f", h=2)
                            e23 = e2[:].rearrange("p (h f) -> p h f", h=2)
                            eb3 = ebt.rearrange("p (h f) -> p h f", h=2)
                            nc.gpsimd.memset(e23[:, :, 0:256], 0.0)
                            nc.scalar.activation(
                                e13[:, :, 256:512], ps3[:, :, 256:512],
                                mybir.ActivationFunctionType.Exp,
                                scale=1.0 / 1024.0)
                            nc.vector.tensor_mul(e23[:, :, 256:512],
                                                 e13[:, :, 256:512],
                                                 eb3[:, :, 256:512])
                        else:
                            nc.scalar.activation(
                                e1[:], ps[:],
                                mybir.ActivationFunctionType.Exp,
                                scale=1.0 / 1024.0)
                            nc.vector.tensor_mul(e2[:], e1[:], ebt)
                        pends.append((e2, jtp, jt_max, po, ir))
                        if len(pends) > 1:
                            stage2(pends.pop(0))
                        yield
                for p_ in pends:
                    stage2(p_)
                pends.clear()

            def run(g):
                for _ in g:
                    pass

            def take(g, n):
                for _ in range(n):
                    try:
                        next(g)
                    except StopIteration:
                        return False
                return True

            def rr2(main, other, ratio=2):
                """Interleave ratio:1; stop when main exhausts (other may
                have leftovers for a later phase)."""
                while True:
                    for _ in range(ratio):
                        if not take(main, 1):
                            return
                    take(other, 1)

            def chain_steps(steps):
                for g, n in steps:
                    for _ in range(n):
                        try:
                            yield next(g)
                        except StopIteration:
                            break
                for g, _ in steps:
                    yield from g

            def zip_gens(*pairs):
                """Round-robin over (gen, per-slot count); yield per round."""
                active = [[g, c] for g, c in pairs]
                while active:
                    for it in list(active):
                        for _ in range(it[1]):
                            try:
                                next(it[0])
                            except StopIteration:
                                active.remove(it)
                                break
                        yield

            def pair(ga, gb):
                """Strict block-by-block interleave of two attention streams."""
                while True:
                    a_ok = take(ga, 1)
                    b_ok = take(gb, 1)
                    if not (a_ok or b_ok):
                        return
                    yield

            p0 = gen_proj(0)
            vt0 = gen_vt(0)
            p1 = gen_proj(1, evac_dve=True)
            v1 = gen_vt(1)
            ebz = chain_steps([(gen_eb(0), 100), (gen_eb(1), 100)])

            # intro: proj-b0 irp0 (v,k,q) + vt jt0-7, eb-h0 trickling on DMA
            for _ in range(3):
                take(p0, 1)
                take(ebz, 1)
            take(vt0, 1)

            # phase C: both b0 attention streams paired, over the rest of
            # b0-proj, all of b1-proj, and wo(b0,0) once deps complete
            paC = pair(gen_attn(0, 0), gen_attn(1, 0))
            bgC = chain_steps([(vt0, 1), (p0, 3), (vt0, 2), (p1, 6), (v1, 4)])
            for _ in range(7):
                take(paC, 1)
                take(bgC, 1)
            w00 = gen_wo(0, 0, evac_dve="alt")
            bgC2 = chain_steps([(bgC, 999), (w00, 100), (ebz, 999)])
            rr2(paC, bgC2, 1)
            run(bgC)   # safety barrier: all projections emitted
            run(w00)

            # phase E: both b1 attention streams over remaining wo work.
            paE = pair(gen_attn(0, 1), gen_attn(1, 1))
            w01 = gen_wo(0, 1, evac_dve="alt")    # deps met (phase C done)
            ww = chain_steps([(w01, 100)])
            for _ in range(14):
                take(paE, 1)
                take(ww, 1)
                if _ >= 7 and _ % 2 == 1:
                    take(ww, 1)
            run(w01)
            w10 = gen_wo(1, 0, evac_dve="alt")    # ir0-1 epilogues done
            for _ in range(4):
                take(paE, 1)
                take(w10, 4)
            run(w10)
            w11 = gen_wo(1, 1, evac_dve="alt")    # half0 needs ir2 (round ~14)
            rr2(paE, chain_steps([(w11, 8), (ebz, 999)]), 1)
            run(w11)

    nc.compile()
    return nc


def _get_nc():
    if "nc" not in _BUILT:
        _BUILT["nc"] = _build()
    return _BUILT["nc"]


def _reference_np(x, attn_bias, mask, gamma, Wq, Wkv, Wo):
    """Numpy fallback (only used if mask is not all-True)."""
    b, n, dim = x.shape
    h, dh = HEADS, DH
    l2 = np.sqrt((x.astype(np.float64) ** 2).sum(-1, keepdims=True))
    xn = x / np.maximum(l2, 1e-12) * (dim ** 0.5) * gamma
    q = (xn @ Wq.T) * (dh ** -0.5)
    kv = xn @ Wkv.T
    k, v = kv[..., :h * dh], kv[..., h * dh:]
    def to_heads(t):
        return t.reshape(b, n, h, dh).transpose(0, 2, 1, 3)
    q, k, v = to_heads(q), to_heads(k), to_heads(v)
    sim = np.einsum('bhid,bhjd->bhij', q, k) + attn_bias
    neg = -np.finfo(np.float32).max
    sim = np.where(mask[:, None, None, :], sim, neg)
    causal = np.triu(np.ones((n, n), dtype=bool), k=1)
    sim = np.where(causal, neg, sim)
    sim = sim - sim.max(-1, keepdims=True)
    e = np.exp(sim)
    attn = e / e.sum(-1, keepdims=True)
    out = np.einsum('bhij,bhjd->bhid', attn, v)
    out = out.transpose(0, 2, 1, 3).reshape(b, n, h * dh)
    return (out @ Wo.T).astype(np.float32)


def kernel(x, attn_bias, mask, gamma, Wq, Wkv, Wo, _trace=False):
    from concourse.bass_utils import run_bass_kernel_spmd

    x = np.asarray(x, dtype=np.float32)
    attn_bias = np.asarray(attn_bias, dtype=np.float32)
    mask = np.asarray(mask)
    gamma = np.asarray(gamma, dtype=np.float32)
    Wq = np.asarray(Wq, dtype=np.float32)
    Wkv = np.asarray(Wkv, dtype=np.float32)
    Wo = np.asarray(Wo, dtype=np.float32)

    if not bool(mask.all()):
        return _reference_np(x, attn_bias, mask, gamma, Wq, Wkv, Wo)

    # ---- host prep (elementwise / layout only) ----
    bf = ml_dtypes.bfloat16
    f8 = ml_dtypes.float8_e4m3
    WS = 32.0   # fp8 weight scale; q,k each carry x32 -> exp scale 1/1024,
                # v carries x32 -> folded into Wo below
    l2 = np.sqrt((x ** 2).sum(-1, keepdims=True))
    xn = x / np.maximum(l2, 1e-12) * (DIM ** 0.5) * gamma
    xnT = np.ascontiguousarray(xn.transpose(0, 2, 1))            # [B, DIM, N]
    xnT8 = xnT.astype(f8)
    xnTc = np.ascontiguousarray(xnT[:, :, :128]).astype(bf)

    tril = np.tril(np.ones((N, N), dtype=np.float32))

    def f8_layout(wT):
        """[DIM,128] -> [128, DIM] DoubleRow image:
        out[p, ((g*2 + kt)*128) + j] = wT[g*256 + kt*128 + p, j]."""
        return np.ascontiguousarray(
            wT.reshape(4, 2, 128, 128).transpose(2, 0, 1, 3).reshape(128, DIM))

    def sbuf_layout(wT):
        """[DIM,128] -> [128, DIM] SBUF image: out[p, c*128+j] = wT[c*128+p, j]."""
        return np.ascontiguousarray(
            wT.reshape(NC_CHUNK, 128, 128).transpose(1, 0, 2).reshape(128, DIM))

    in_maps = []
    for c in range(N_CORES):
        r0 = c * 128
        wq_c = f8_layout((Wq[r0:r0 + 128] * (WS * DH ** -0.5)).T).astype(f8)
        wk_c = f8_layout((Wkv[r0:r0 + 128] * WS).T).astype(f8)
        wv_c = f8_layout(
            (Wkv[HEADS * DH + r0:HEADS * DH + r0 + 128] * WS).T).astype(f8)
        wvc_c = sbuf_layout(
            (Wkv[HEADS * DH + r0:HEADS * DH + r0 + 128] * WS).T).astype(bf)
        wo_c = np.ascontiguousarray(
            (Wo[:, r0:r0 + 128] / WS).T).astype(bf)  # [128, DIM]
        bias_c = attn_bias[H_LOC * c:H_LOC * (c + 1)]            # [2, N, N]
        ebc = np.exp(bias_c) * tril                              # mask j>i
        ebc = np.ascontiguousarray(ebc.transpose(0, 2, 1)).astype(bf)  # [h,j,i]
        in_maps.append({"xnT": xnT8, "xnTc": xnTc, "wq": wq_c, "wk": wk_c,
                        "wv": wv_c, "wvc": wvc_c, "wo": wo_c, "eb": ebc})

    nc = _get_nc()
    try:
        res = run_bass_kernel_spmd(nc, in_maps,
                                   core_ids=list(range(N_CORES)),
                                   trace=_trace)
    except ModuleNotFoundError:
        # NTFF profiling hook unavailable in this environment
        res = run_bass_kernel_spmd(nc, in_maps,
                                   core_ids=list(range(N_CORES)))
    acc = res.results[0]["o"].astype(np.float32)
    for c in range(1, N_CORES):
        acc += res.results[c]["o"].astype(np.float32)
    out = np.ascontiguousarray(acc.transpose(0, 2, 1))           # [B, N, DIM]
    if _trace:
        kernel._last_results = res
    return out


# revision 95
# speedup vs baseline: 1.0871x; 1.0313x over previous
"""Trainium2 Bass kernel for dense-transformer attention block.

Reference computation (see harness):
  xn  = x / max(||x||_2, 1e-12) * sqrt(dim) * gamma          (RMSNorm-as-written)
  q   = (xn @ Wq.T) * dh^-0.5 ; k, v = split(xn @ Wkv.T)
  sim = q k^T + attn_bias ; key-pad mask ; causal mask
  out = softmax(sim) @ v @ Wo.T

Sharding: 16 heads / 8 cores = 2 heads per core (tensor parallel).
Each core computes its 2 heads' attention + its column-slice of Wo,
producing a partial output; host sums the 8 partials.

Device dataflow (per core), everything in transposed token-on-free layout:
  qT/kT/vT = W^T-stationary matmuls over xnT (bf16, N=512)
  v        = PE-transpose of vT, + ones column (softmax denominator trick)
  S^T      = kT.T-slices @ qT  (per head, causal-triangular blocks only)
  E        = exp(S^T) * exp_bias_T   (exp(bias) precomputed on host, causal-
             masked there; softmax needs no max-subtraction: |logits| < ~15)
  O^T      = v' stationary @ E  -> row 64 = denominator
  o2       = O^T[0:64] * broadcast(1/denom)   (folded into the po evacuation)
  out^T    = WoT-chunk stationary @ o2
Host prep: RMSNorm + transposes + weight folding + exp(bias) (elementwise);
all GEMMs and softmax run on device. All HBM traffic is bf16.
"""
import sys
import numpy as np

sys.path.insert(0, "/opt/trn_rl_repo")

import ml_dtypes  # noqa: E402

N_CORES = 8
B = 2
N = 2048
DIM = 1024
HEADS = 16
DH = 64
H_LOC = HEADS // N_CORES  # 2 heads per core
NT = N // 128             # 16 token tiles of 128
NIR = N // 512            # 4 i-ranges of 512
NC_CHUNK = DIM // 128     # 8 contraction chunks

_BUILT = {}


def _build():
    """Construct + compile the per-core Bass program (same for all cores)."""
    import concourse.bass as bass
    import concourse.mybir as mybir
    from concourse import bacc
    from concourse.tile import TileContext
    from concourse.masks import make_identity

    f32 = mybir.dt.float32
    bf16 = mybir.dt.bfloat16
    f8 = mybir.dt.float8e4

    nc = bacc.Bacc("TRN2", target_bir_lowering=False, debug=False,
                   num_devices=N_CORES)

    xnT = nc.dram_tensor("xnT", [B, DIM, N], f8, kind="ExternalInput").ap()
    xnTc = nc.dram_tensor("xnTc", [B, DIM, 128], bf16, kind="ExternalInput").ap()
    # q/k/v weights in fp8 DoubleRow SBUF layout (x32); wvc = bf16 x32 copy
    # of wv for the tokens 0-127 correction patch (early causal rows see few
    # keys, so v quantization error there passes straight to the output)
    wq = nc.dram_tensor("wq", [128, DIM], f8, kind="ExternalInput").ap()
    wk = nc.dram_tensor("wk", [128, DIM], f8, kind="ExternalInput").ap()
    wv = nc.dram_tensor("wv", [128, DIM], f8, kind="ExternalInput").ap()
    wvc = nc.dram_tensor("wvc", [128, DIM], bf16, kind="ExternalInput").ap()
    wo = nc.dram_tensor("wo", [128, DIM], bf16, kind="ExternalInput").ap()
    eb = nc.dram_tensor("eb", [H_LOC, N, N], bf16, kind="ExternalInput").ap()
    o = nc.dram_tensor("o", [B, DIM, N], bf16, kind="ExternalOutput").ap()

    with TileContext(nc) as tc:
        with (
            tc.tile_pool(name="const", bufs=1) as constp,
            tc.tile_pool(name="wts", bufs=1) as wtp,
            tc.tile_pool(name="xn", bufs=4) as xnp,
            tc.tile_pool(name="qkv", bufs=1) as qkvp,
            tc.tile_pool(name="ebp", bufs=40) as ebpool,
            tc.tile_pool(name="ework", bufs=4) as ework,
            tc.tile_pool(name="o2", bufs=1) as o2p,
            tc.tile_pool(name="dance", bufs=3) as dancep,
            tc.tile_pool(name="outs", bufs=9) as outsp,
            tc.tile_pool(name="qkh", bufs=2, space="PSUM") as psA,
            tc.tile_pool(name="mmhalf", bufs=2, space="PSUM") as psD,
            tc.tile_pool(name="pacc", bufs=2, space="PSUM") as psB,
        ):
            ident = constp.tile([128, 128], f32)
            make_identity(nc, ident[:])
            identb = constp.tile([128, 128], bf16, name="identb")
            nc.vector.tensor_copy(identb[:], ident[:])

            wq_sb = wtp.tile([128, NC_CHUNK * 128], f8, tag="w1")
            wk_sb = wtp.tile([128, NC_CHUNK * 128], f8, tag="wk")
            wv_sb = wtp.tile([128, NC_CHUNK * 128], f8, tag="wv")
            wvc_sb = wtp.tile([128, NC_CHUNK * 128], bf16, tag="wvc")
            wo_sb = wtp.tile([128, DIM], bf16, tag="wo", name="wo_sb")
            # load order matches first use: v, k, q (proj runs v first)
            for wsb_, wdr_ in ((wv_sb, wv), (wk_sb, wk), (wq_sb, wq),
                               (wo_sb, wo), (wvc_sb, wvc)):
                nc.scalar.dma_start(out=wsb_[:], in_=wdr_[:])

            qT = [qkvp.tile([128, N], bf16, tag=f"qT{b}", name=f"qT{b}")
                  for b in range(B)]
            kT = [qkvp.tile([128, N], bf16, tag=f"kT{b}", name=f"kT{b}")
                  for b in range(B)]
            vT = [qkvp.tile([128, N], bf16, tag=f"vT{b}", name=f"vT{b}")
                  for b in range(B)]
            vn = [qkvp.tile([128, H_LOC * NT * 65], bf16, tag=f"vn{b}",
                            name=f"vn{b}") for b in range(B)]
            o2 = [o2p.tile([128, N], bf16, tag=f"o2{b}", name=f"o2{b}")
                  for b in range(B)]
            # ones columns of vn (every 65th col), written once per batch
            for b in range(B):
                nc.vector.memset(
                    vn[b][:].rearrange("p (t c) -> p t c", c=65)[:, :, 64:65],
                    1.0)

            def gen_proj(b, evac_dve=False):
                # all projections via fp8 DoubleRow (4 chunk-pairs, x32
                # weights); v gets a bf16 correction for tokens 0-127
                for irp in range(NIR // 2):
                    if b == 0 and irp == 0:
                        x8t = [xnp.tile([128, 2 * 1024], f8, tag="x8", bufs=4,
                                        name=f"x8{b}_{irp}_{g}")
                               for g in range(4)]
                        x8 = [t[:] for t in x8t]
                        for g in range(4):
                            nc.sync.dma_start(
                                out=x8[g].rearrange("p (kt f) -> p kt f",
                                                    kt=2),
                                in_=xnT[b, g * 256:(g + 1) * 256,
                                        irp * 1024:(irp + 1) * 1024].rearrange(
                                            "(kt p) f -> p kt f", kt=2))
                    else:
                        xw = xnp.tile([128, 8 * 1024], f8, tag="x8w", bufs=2,
                                      name=f"x8w{b}_{irp}")
                        nc.sync.dma_start(
                            out=xw[:].rearrange("p (g kt f) -> p g kt f",
                                                g=4, kt=2),
                            in_=xnT[b, :, irp * 1024:(irp + 1) * 1024].rearrange(
                                "(g kt p) f -> p g kt f", g=4, kt=2))
                        x8 = [xw[:, g * 2048:(g + 1) * 2048] for g in range(4)]
                    if irp == 0:
                        xc = xnp.tile([128, NC_CHUNK * 128], bf16, tag="xc",
                                      bufs=2, name=f"xc{b}")
                        nc.sync.dma_start(
                            out=xc[:].rearrange("p (c f) -> p c f", c=NC_CHUNK),
                            in_=xnTc[b].rearrange("(c p) f -> p c f",
                                                  c=NC_CHUNK))
                    for wsb, dst in ((wv_sb, vT[b]), (wk_sb, kT[b]),
                                     (wq_sb, qT[b])):
                        wsb4 = wsb[:].rearrange("p (g kt j) -> p g kt j",
                                                g=4, kt=2)
                        for half in range(2):
                            p = psD.tile([128, 512], f32, tag="mm",
                                         name=f"pp{b}_{irp}_{half}")
                            for g in range(4):
                                nc.tensor.matmul(
                                    p[:],
                                    wsb4[:, g],
                                    x8[g].rearrange(
                                        "p (kt f) -> p kt f", kt=2)[
                                        :, :, half * 512:(half + 1) * 512],
                                    start=(g == 0), stop=(g == 3),
                                    perf_mode=mybir.MatmulPerfMode.DoubleRow)
                            sl = dst[:, irp * 1024 + half * 512:
                                     irp * 1024 + (half + 1) * 512]
                            if (evac_dve or irp == 1) and half == 0:
                                nc.vector.tensor_copy(sl, p[:])
                            else:
                                nc.scalar.copy(sl, p[:])
                        if wsb is wv_sb and irp == 0:
                            # bf16 recompute of v for tokens 0-127
                            pc = psD.tile([128, 128], f32, tag="mm",
                                          name=f"pvc{b}")
                            for c in range(NC_CHUNK):
                                nc.tensor.matmul(
                                    pc[:],
                                    wvc_sb[:, c * 128:(c + 1) * 128],
                                    xc[:, c * 128:(c + 1) * 128],
                                    start=(c == 0), stop=(c == NC_CHUNK - 1))
                            if evac_dve:
                                nc.vector.tensor_copy(vT[b][:, 0:128], pc[:])
                            else:
                                nc.scalar.copy(vT[b][:, 0:128], pc[:])
                        yield

            def gen_vt(b):
                for jt in range(NT):
                    pv = psD.tile([128, 128], bf16, tag="mm",
                                  name=f"pv{b}_{jt}")
                    nc.tensor.matmul(pv[:], vT[b][:, jt * 128:(jt + 1) * 128],
                                     identb[:], is_transpose=True)
                    for h in range(H_LOC):
                        base = (h * NT + jt) * 65
                        if b == 0 and jt < 8:
                            nc.scalar.copy(vn[b][:, base:base + 64],
                                           pv[:, h * 64:h * 64 + 64])
                        else:
                            nc.vector.tensor_copy(vn[b][:, base:base + 64],
                                                  pv[:, h * 64:h * 64 + 64])
                    if jt % 4 == 3:
                        yield

            def gen_wo(b, irp, evac_dve=True):
                # per-(half, ec) steps, half-major; stage-pipelined evac;
                # one output DMA per ec (fires after the half-1 evacuation)
                pend = None
                osb_tiles = {}

                def evac(p):
                    pw, ec, half, i = p
                    if ec not in osb_tiles:
                        osb_tiles[ec] = outsp.tile(
                            [128, 1024], bf16, tag="osb",
                            name=f"osb{b}_{ec}_{irp}")
                    osb = osb_tiles[ec]
                    dst = osb[:, half * 512:(half + 1) * 512]
                    if evac_dve is True or (evac_dve == "alt" and (ec + half) % 2 == 0):
                        nc.vector.tensor_copy(dst, pw[:])
                    else:
                        nc.scalar.copy(dst, pw[:])
                    if half == 1:
                        nc.sync.dma_start(
                            out=o[b, ec * 128:(ec + 1) * 128,
                                  irp * 1024:(irp + 1) * 1024],
                            in_=osb[:])

                i = 0
                for half in range(2):
                    for ec in range(NC_CHUNK):
                        pw = psD.tile([128, 512], f32, tag="mm",
                                      name=f"pw{b}_{ec}_{irp}_{half}")
                        nc.tensor.matmul(
                            pw[:],
                            wo_sb[:, ec * 128:(ec + 1) * 128],
                            o2[b][:, irp * 1024 + half * 512:
                                  irp * 1024 + (half + 1) * 512],
                            start=True, stop=True)
                        if pend is not None:
                            evac(pend)
                        pend = (pw, ec, half, i)
                        i += 1
                        yield
                evac(pend)

            eb_tiles = {}

            def emit_eb_ir(h, ir):
                if (h, ir) in eb_tiles:
                    return
                njt = min(4 * ir + 4, NT)
                t = ebpool.tile([128, njt * 512], bf16, tag=f"eb{ir}", bufs=2,
                                name=f"eb_{h}_{ir}")
                nc.sync.dma_start(
                    out=t[:].rearrange("p (j f) -> p j f", j=njt),
                    in_=eb[h, 0:njt * 128, ir * 512:(ir + 1) * 512].rearrange(
                        "(j p) f -> p j f", j=njt))
                eb_tiles[(h, ir)] = t

            def emit_eb_tile(h, jtp, ir):
                emit_eb_ir(h, ir)

            def gen_eb(h):
                for ir in range(NIR):
                    emit_eb_ir(h, ir)
                    yield

            def gen_attn(h, b):
                # stage-pipelined: PV (+ epilogue at ir close) for block k-1
                # is emitted after QK/exp/mul of block k, so no engine queue
                # entry ever waits long (avoids head-of-line blocking).
                pends = []

                def epilogue(po, ir):
                    rfl = dancep.tile([1, 512], bf16, tag="rfl", bufs=2,
                                      name=f"rf{h}_{b}_{ir}")
                    with nc.allow_low_precision(reason="bf16 1/denom"):
                        nc.vector.reciprocal(rfl[0:1, :], po[64:65, :])
                    rb = ework.tile([128, 512], bf16, tag="rb", bufs=2,
                                    name=f"rb{h}_{b}_{ir}")
                    for s in range(4):
                        nc.gpsimd.partition_broadcast(
                            rb[:, s * 128:(s + 1) * 128],
                            rfl[0:1, s * 128:(s + 1) * 128])
                    if h == 0:
                        nc.vector.tensor_mul(
                            o2[b][0:64, ir * 512:(ir + 1) * 512],
                            po[0:64, :], rb[0:64, :])
                    else:
                        tmp = ework.tile([64, 512], bf16, tag="hshift",
                                         bufs=2, name=f"hs{h}_{b}_{ir}")
                        nc.vector.tensor_mul(tmp[:], po[0:64, :], rb[0:64, :])
                        nc.sync.dma_start(
                            out=o2[b][64:128, ir * 512:(ir + 1) * 512],
                            in_=tmp[:])

                def stage2(p):
                    e2, jtp, jt_max, po, ir = p
                    for half in range(2):
                        jt = 2 * jtp + half
                        vbase = (h * NT + jt) * 65
                        nc.tensor.matmul(
                            po[:], vn[b][:, vbase:vbase + 65],
                            e2[:, half * 512:(half + 1) * 512],
                            start=(jt == 0), stop=(jt == jt_max - 1))
                    if 2 * jtp + 1 == jt_max - 1:
                        epilogue(po, ir)

                for ir in range(NIR):
                    jt_max = min(4 * ir + 4, NT)
                    po = psB.tile([65, 512], f32, tag="po",
                                  name=f"po{h}_{b}_{ir}")
                    for jtp in range(jt_max // 2):
                        emit_eb_ir(h, ir)
                        ebt = eb_tiles[(h, ir)][:, jtp * 1024:(jtp + 1) * 1024]
                        # last jtp of each ir is the diagonal pair: j > i for
                        # all i in the first half of the i-range, so compute
                        # only i in [256, 512) and zero-fill the rest of e2
                        diag = (jtp == jt_max // 2 - 1)
                        i0 = 256 if diag else 0
                        ps = psA.tile([128, 1024], f32, tag="qk",
                                      name=f"ps{h}_{b}_{ir}_{jtp}")
                        for half in range(2):
                            jt = 2 * jtp + half
                            nc.tensor.matmul(
                                ps[:, half * 512 + i0:(half + 1) * 512],
                                kT[b][h * 64:(h + 1) * 64,
                                      jt * 128:(jt + 1) * 128],
                                qT[b][h * 64:(h + 1) * 64,
                                      ir * 512 + i0:(ir + 1) * 512],
                                start=True, stop=True)
                        e1 = ework.tile([128, 1024], bf16, tag="e1", bufs=6,
                                        name=f"e1_{h}_{b}_{ir}_{jtp}")
                        e2 = ework.tile([128, 1024], bf16, tag="e2", bufs=6,
                                        name=f"e2_{h}_{b}_{ir}_{jtp}")
                        if diag:
                            ps3 = ps[:].rearrange("p (h f) -> p h f", h=2)
                            e13 = e1[:].rearrange("p (h f) -> p h f", h=2)
                            e23 = e2[:].rearrange("p (h f) -> p h f", h=2)
                            eb3 = ebt.rearrange("p (h f) -> p h f", h=2)
                            nc.gpsimd.memset(e23[:, :, 0:256], 0.0)
                            nc.scalar.activation(
                                e13[:, :, 256:512], ps3[:, :, 256:512],
                                mybir.ActivationFunctionType.Exp,
                                scale=1.0 / 1024.0)
                            nc.vector.tensor_mul(e23[:, :, 256:512],
                                                 e13[:, :, 256:512],
                                                 eb3[:, :, 256:512])
                        else:
                            nc.scalar.activation(
                                e1[:], ps[:],
                                mybir.ActivationFunctionType.Exp,
                                scale=1.0 / 1024.0)
                            nc.vector.tensor_mul(e2[:], e1[:], ebt)
                        pends.append((e2, jtp, jt_max, po, ir))
                        if len(pends) > 1:
                            stage2(pends.pop(0))
                        yield
                for p_ in pends:
                    stage2(p_)
                pends.clear()

            def run(g):
                for _ in g:
                    pass

            def take(g, n):
                for _ in range(n):
                    try:
                        next(g)
                    except StopIteration:
                        return False
                return True

            def rr2(main, other, ratio=2):
                """Interleave ratio:1; stop when main exhausts (other may
                have leftovers for a later phase)."""
                while True:
                    for _ in range(ratio):
                        if not take(main, 1):
                            return
                    take(other, 1)

            def chain_steps(steps):
                for g, n in steps:
                    for _ in range(n):
                        try:
                            yield next(g)
                        except StopIteration:
                            break
                for g, _ in steps:
                    yield from g

            def zip_gens(*pairs):
                """Round-robin over (gen, per-slot count); yield per round."""
                active = [[g, c] for g, c in pairs]
                while active:
                    for it in list(active):
                        for _ in range(it[1]):
                            try:
                                next(it[0])
                            except StopIteration:
                                active.remove(it)
                                break
                        yield

            def pair(ga, gb):
                """Interleave two attention streams, 2 blocks per turn."""
                while True:
                    a_ok = take(ga, 2)
                    b_ok = take(gb, 2)
                    if not (a_ok or b_ok):
                        return
                    yield
                    yield

            p0 = gen_proj(0)
            vt0 = gen_vt(0)
            p1 = gen_proj(1, evac_dve=True)
            v1 = gen_vt(1)
            ebz = chain_steps([(gen_eb(0), 100), (gen_eb(1), 100)])

            # intro: proj-b0 irp0 (v,k,q) + vt jt0-7, eb-h0 trickling on DMA
            for _ in range(3):
                take(p0, 1)
                take(ebz, 1)
            take(vt0, 1)

            # phase C: both b0 attention streams paired, over the rest of
            # b0-proj, all of b1-proj, and wo(b0,0) once deps complete
            paC = pair(gen_attn(0, 0), gen_attn(1, 0))
            bgC = chain_steps([(vt0, 1), (p0, 3), (vt0, 2), (p1, 6), (v1, 4)])
            for _ in range(7):
                take(paC, 1)
                take(bgC, 1)
            w00 = gen_wo(0, 0, evac_dve="alt")
            bgC2 = chain_steps([(bgC, 999), (w00, 100), (ebz, 999)])
            rr2(paC, bgC2, 1)
            run(bgC)   # safety barrier: all projections emitted
            run(w00)

            # phase E: both b1 attention streams over remaining wo work.
            paE = pair(gen_attn(0, 1), gen_attn(1, 1))
            w01 = gen_wo(0, 1, evac_dve="alt")    # deps met (phase C done)
            ww = chain_steps([(w01, 100)])
            for _ in range(14):
                take(paE, 1)
                take(ww, 1)
                if _ >= 7 and _ % 2 == 1:
                    take(ww, 1)
            run(w01)
            w10 = gen_wo(1, 0, evac_dve="alt")    # ir0-1 epilogues done
            for _ in range(4):
                take(paE, 1)
                take(w10, 4)
            run(w10)
            w11 = gen_wo(1, 1, evac_dve="alt")    # half0 needs ir2 (round ~14)
            rr2(paE, chain_steps([(w11, 8), (ebz, 999)]), 1)
            run(w11)

    nc.compile()
    return nc


def _get_nc():
    if "nc" not in _BUILT:
        _BUILT["nc"] = _build()
    return _BUILT["nc"]


def _reference_np(x, attn_bias, mask, gamma, Wq, Wkv, Wo):
    """Numpy fallback (only used if mask is not all-True)."""
    b, n, dim = x.shape
    h, dh = HEADS, DH
    l2 = np.sqrt((x.astype(np.float64) ** 2).sum(-1, keepdims=True))
    xn = x / np.maximum(l2, 1e-12) * (dim ** 0.5) * gamma
    q = (xn @ Wq.T) * (dh ** -0.5)
    kv = xn @ Wkv.T
    k, v = kv[..., :h * dh], kv[..., h * dh:]
    def to_heads(t):
        return t.reshape(b, n, h, dh).transpose(0, 2, 1, 3)
    q, k, v = to_heads(q), to_heads(k), to_heads(v)
    sim = np.einsum('bhid,bhjd->bhij', q, k) + attn_bias
    neg = -np.finfo(np.float32).max
    sim = np.where(mask[:, None, None, :], sim, neg)
    causal = np.triu(np.ones((n, n), dtype=bool), k=1)
    sim = np.where(causal, neg, sim)
    sim = sim - sim.max(-1, keepdims=True)
    e = np.exp(sim)
    attn = e / e.sum(-1, keepdims=True)
    out = np.einsum('bhij,bhjd->bhid', attn, v)
    out = out.transpose(0, 2, 1, 3).reshape(b, n, h * dh)
    return (out @ Wo.T).astype(np.float32)


def kernel(x, attn_bias, mask, gamma, Wq, Wkv, Wo, _trace=False):
    from concourse.bass_utils import run_bass_kernel_spmd

    x = np.asarray(x, dtype=np.float32)
    attn_bias = np.asarray(attn_bias, dtype=np.float32)
    mask = np.asarray(mask)
    gamma = np.asarray(gamma, dtype=np.float32)
    Wq = np.asarray(Wq, dtype=np.float32)
    Wkv = np.asarray(Wkv, dtype=np.float32)
    Wo = np.asarray(Wo, dtype=np.float32)

    if not bool(mask.all()):
        return _reference_np(x, attn_bias, mask, gamma, Wq, Wkv, Wo)

    # ---- host prep (elementwise / layout only) ----
    bf = ml_dtypes.bfloat16
    f8 = ml_dtypes.float8_e4m3
    WS = 32.0   # fp8 weight scale; q,k each carry x32 -> exp scale 1/1024,
                # v carries x32 -> folded into Wo below
    l2 = np.sqrt((x ** 2).sum(-1, keepdims=True))
    xn = x / np.maximum(l2, 1e-12) * (DIM ** 0.5) * gamma
    xnT = np.ascontiguousarray(xn.transpose(0, 2, 1))            # [B, DIM, N]
    xnT8 = xnT.astype(f8)
    xnTc = np.ascontiguousarray(xnT[:, :, :128]).astype(bf)

    tril = np.tril(np.ones((N, N), dtype=np.float32))

    def f8_layout(wT):
        """[DIM,128] -> [128, DIM] DoubleRow image:
        out[p, ((g*2 + kt)*128) + j] = wT[g*256 + kt*128 + p, j]."""
        return np.ascontiguousarray(
            wT.reshape(4, 2, 128, 128).transpose(2, 0, 1, 3).reshape(128, DIM))

    def sbuf_layout(wT):
        """[DIM,128] -> [128, DIM] SBUF image: out[p, c*128+j] = wT[c*128+p, j]."""
        return np.ascontiguousarray(
            wT.reshape(NC_CHUNK, 128, 128).transpose(1, 0, 2).reshape(128, DIM))

    in_maps = []
    for c in range(N_CORES):
        r0 = c * 128
        wq_c = f8_layout((Wq[r0:r0 + 128] * (WS * DH ** -0.5)).T).astype(f8)
        wk_c = f8_layout((Wkv[r0:r0 + 128] * WS).T).astype(f8)
        wv_c = f8_layout(
            (Wkv[HEADS * DH + r0:HEADS * DH + r0 + 128] * WS).T).astype(f8)
        wvc_c = sbuf_layout(
            (Wkv[HEADS * DH + r0:HEADS * DH + r0 + 128] * WS).T).astype(bf)
        wo_c = np.ascontiguousarray(
            (Wo[:, r0:r0 + 128] / WS).T).astype(bf)  # [128, DIM]
        bias_c = attn_bias[H_LOC * c:H_LOC * (c + 1)]            # [2, N, N]
        ebc = np.exp(bias_c) * tril                              # mask j>i
        ebc = np.ascontiguousarray(ebc.transpose(0, 2, 1)).astype(bf)  # [h,j,i]
        in_maps.append({"xnT": xnT8, "xnTc": xnTc, "wq": wq_c, "wk": wk_c,
                        "wv": wv_c, "wvc": wvc_c, "wo": wo_c, "eb": ebc})

    nc = _get_nc()
    try:
        res = run_bass_kernel_spmd(nc, in_maps,
                                   core_ids=list(range(N_CORES)),
                                   trace=_trace)
    except ModuleNotFoundError:
        # NTFF profiling hook unavailable in this environment
        res = run_bass_kernel_spmd(nc, in_maps,
                                   core_ids=list(range(N_CORES)))
    acc = res.results[0]["o"].astype(np.float32)
    for c in range(1, N_CORES):
        acc += res.results[c]["o"].astype(np.float32)
    out = np.ascontiguousarray(acc.transpose(0, 2, 1))           # [B, N, DIM]
    if _trace:
        kernel._last_results = res
    return out


# revision 98
# speedup vs baseline: 1.0982x; 1.0102x over previous
"""Trainium2 Bass kernel for dense-transformer attention block.

Reference computation (see harness):
  xn  = x / max(||x||_2, 1e-12) * sqrt(dim) * gamma          (RMSNorm-as-written)
  q   = (xn @ Wq.T) * dh^-0.5 ; k, v = split(xn @ Wkv.T)
  sim = q k^T + attn_bias ; key-pad mask ; causal mask
  out = softmax(sim) @ v @ Wo.T

Sharding: 16 heads / 8 cores = 2 heads per core (tensor parallel).
Each core computes its 2 heads' attention + its column-slice of Wo,
producing a partial output; host sums the 8 partials.

Device dataflow (per core), everything in transposed token-on-free layout:
  qT/kT/vT = W^T-stationary matmuls over xnT (bf16, N=512)
  v        = PE-transpose of vT, + ones column (softmax denominator trick)
  S^T      = kT.T-slices @ qT  (per head, causal-triangular blocks only)
  E        = exp(S^T) * exp_bias_T   (exp(bias) precomputed on host, causal-
             masked there; softmax needs no max-subtraction: |logits| < ~15)
  O^T      = v' stationary @ E  -> row 64 = denominator
  o2       = O^T[0:64] * broadcast(1/denom)   (folded into the po evacuation)
  out^T    = WoT-chunk stationary @ o2
Host prep: RMSNorm + transposes + weight folding + exp(bias) (elementwise);
all GEMMs and softmax run on device. All HBM traffic is bf16.
"""
import sys
import numpy as np

sys.path.insert(0, "/opt/trn_rl_repo")

import ml_dtypes  # noqa: E402

N_CORES = 8
B = 2
N = 2048
DIM = 1024
HEADS = 16
DH = 64
H_LOC = HEADS // N_CORES  # 2 heads per core
NT = N // 128             # 16 token tiles of 128
NIR = N // 512            # 4 i-ranges of 512
NC_CHUNK = DIM // 128     # 8 contraction chunks

_BUILT = {}


def _build():
    """Construct + compile the per-core Bass program (same for all cores)."""
    import concourse.bass as bass
    import concourse.mybir as mybir
    from concourse import bacc
    from concourse.tile import TileContext
    from concourse.masks import make_identity

    f32 = mybir.dt.float32
    bf16 = mybir.dt.bfloat16
    f8 = mybir.dt.float8e4

    nc = bacc.Bacc("TRN2", target_bir_lowering=False, debug=False,
                   num_devices=N_CORES)

    xnT = nc.dram_tensor("xnT", [B, DIM, N], f8, kind="ExternalInput").ap()
    xnTc = nc.dram_tensor("xnTc", [B, DIM, 128], bf16, kind="ExternalInput").ap()
    # q/k/v weights in fp8 DoubleRow SBUF layout (x32); wvc = bf16 x32 copy
    # of wv for the tokens 0-127 correction patch (early causal rows see few
    # keys, so v quantization error there passes straight to the output)
    wq = nc.dram_tensor("wq", [128, DIM], f8, kind="ExternalInput").ap()
    wk = nc.dram_tensor("wk", [128, DIM], f8, kind="ExternalInput").ap()
    wv = nc.dram_tensor("wv", [128, DIM], f8, kind="ExternalInput").ap()
    wvc = nc.dram_tensor("wvc", [128, DIM], bf16, kind="ExternalInput").ap()
    wo = nc.dram_tensor("wo", [128, DIM], bf16, kind="ExternalInput").ap()
    eb = nc.dram_tensor("eb", [H_LOC, N, N], bf16, kind="ExternalInput").ap()
    o = nc.dram_tensor("o", [B, DIM, N], bf16, kind="ExternalOutput").ap()

    with TileContext(nc) as tc:
        with (
            tc.tile_pool(name="const", bufs=1) as constp,
            tc.tile_pool(name="wts", bufs=1) as wtp,
            tc.tile_pool(name="xn", bufs=4) as xnp,
            tc.tile_pool(name="qkv", bufs=1) as qkvp,
            tc.tile_pool(name="ebp", bufs=40) as ebpool,
            tc.tile_pool(name="ework", bufs=4) as ework,
            tc.tile_pool(name="o2", bufs=1) as o2p,
            tc.tile_pool(name="dance", bufs=3) as dancep,
            tc.tile_pool(name="outs", bufs=9) as outsp,
            tc.tile_pool(name="qkh", bufs=2, space="PSUM") as psA,
            tc.tile_pool(name="mmhalf", bufs=2, space="PSUM") as psD,
            tc.tile_pool(name="pacc", bufs=2, space="PSUM") as psB,
        ):
            ident = constp.tile([128, 128], f32)
            make_identity(nc, ident[:])
            identb = constp.tile([128, 128], bf16, name="identb")
            nc.vector.tensor_copy(identb[:], ident[:])

            wq_sb = wtp.tile([128, NC_CHUNK * 128], f8, tag="w1")
            wk_sb = wtp.tile([128, NC_CHUNK * 128], f8, tag="wk")
            wv_sb = wtp.tile([128, NC_CHUNK * 128], f8, tag="wv")
            wvc_sb = wtp.tile([128, NC_CHUNK * 128], bf16, tag="wvc")
            wo_sb = wtp.tile([128, DIM], bf16, tag="wo", name="wo_sb")
            # load order matches first use: v, k, q (proj runs v first)
            for wsb_, wdr_ in ((wv_sb, wv), (wk_sb, wk), (wq_sb, wq),
                               (wo_sb, wo), (wvc_sb, wvc)):
                nc.scalar.dma_start(out=wsb_[:], in_=wdr_[:])

            qT = [qkvp.tile([128, N], bf16, tag=f"qT{b}", name=f"qT{b}")
                  for b in range(B)]
            kT = [qkvp.tile([128, N], bf16, tag=f"kT{b}", name=f"kT{b}")
                  for b in range(B)]
            vT = [qkvp.tile([128, N], bf16, tag=f"vT{b}", name=f"vT{b}")
                  for b in range(B)]
            vn = [qkvp.tile([128, H_LOC * NT * 65], bf16, tag=f"vn{b}",
                            name=f"vn{b}") for b in range(B)]
            o2 = [o2p.tile([128, N], bf16, tag=f"o2{b}", name=f"o2{b}")
                  for b in range(B)]
            # ones columns of vn (every 65th col), written once per batch
            for b in range(B):
                nc.vector.memset(
                    vn[b][:].rearrange("p (t c) -> p t c", c=65)[:, :, 64:65],
                    1.0)

            def gen_proj(b, evac_dve=False):
                # all projections via fp8 DoubleRow (4 chunk-pairs, x32
                # weights); v gets a bf16 correction for tokens 0-127
                for irp in range(NIR // 2):
                    if b == 0 and irp == 0:
                        x8t = [xnp.tile([128, 2 * 1024], f8, tag="x8", bufs=4,
                                        name=f"x8{b}_{irp}_{g}")
                               for g in range(4)]
                        x8 = [t[:] for t in x8t]
                        for g in range(4):
                            nc.sync.dma_start(
                                out=x8[g].rearrange("p (kt f) -> p kt f",
                                                    kt=2),
                                in_=xnT[b, g * 256:(g + 1) * 256,
                                        irp * 1024:(irp + 1) * 1024].rearrange(
                                            "(kt p) f -> p kt f", kt=2))
                    else:
                        xw = xnp.tile([128, 8 * 1024], f8, tag="x8w", bufs=2,
                                      name=f"x8w{b}_{irp}")
                        nc.sync.dma_start(
                            out=xw[:].rearrange("p (g kt f) -> p g kt f",
                                                g=4, kt=2),
                            in_=xnT[b, :, irp * 1024:(irp + 1) * 1024].rearrange(
                                "(g kt p) f -> p g kt f", g=4, kt=2))
                        x8 = [xw[:, g * 2048:(g + 1) * 2048] for g in range(4)]
                    if irp == 0:
                        xc = xnp.tile([128, NC_CHUNK * 128], bf16, tag="xc",
                                      bufs=2, name=f"xc{b}")
                        nc.sync.dma_start(
                            out=xc[:].rearrange("p (c f) -> p c f", c=NC_CHUNK),
                            in_=xnTc[b].rearrange("(c p) f -> p c f",
                                                  c=NC_CHUNK))
                    for wsb, dst in ((wv_sb, vT[b]), (wk_sb, kT[b]),
                                     (wq_sb, qT[b])):
                        wsb4 = wsb[:].rearrange("p (g kt j) -> p g kt j",
                                                g=4, kt=2)
                        for half in range(2):
                            p = psD.tile([128, 512], f32, tag="mm",
                                         name=f"pp{b}_{irp}_{half}")
                            for g in range(4):
                                nc.tensor.matmul(
                                    p[:],
                                    wsb4[:, g],
                                    x8[g].rearrange(
                                        "p (kt f) -> p kt f", kt=2)[
                                        :, :, half * 512:(half + 1) * 512],
                                    start=(g == 0), stop=(g == 3),
                                    perf_mode=mybir.MatmulPerfMode.DoubleRow)
                            sl = dst[:, irp * 1024 + half * 512:
                                     irp * 1024 + (half + 1) * 512]
                            if (evac_dve or irp == 1) and half == 0:
                                nc.vector.tensor_copy(sl, p[:])
                            else:
                                nc.scalar.copy(sl, p[:])
                        if wsb is wv_sb and irp == 0:
                            # bf16 recompute of v for tokens 0-127
                            pc = psD.tile([128, 128], f32, tag="mm",
                                          name=f"pvc{b}")
                            for c in range(NC_CHUNK):
                                nc.tensor.matmul(
                                    pc[:],
                                    wvc_sb[:, c * 128:(c + 1) * 128],
                                    xc[:, c * 128:(c + 1) * 128],
                                    start=(c == 0), stop=(c == NC_CHUNK - 1))
                            if evac_dve:
                                nc.vector.tensor_copy(vT[b][:, 0:128], pc[:])
                            else:
                                nc.scalar.copy(vT[b][:, 0:128], pc[:])
                        yield

            def gen_vt(b):
                for jt in range(NT):
                    pv = psD.tile([128, 128], bf16, tag="mm",
                                  name=f"pv{b}_{jt}")
                    nc.tensor.matmul(pv[:], vT[b][:, jt * 128:(jt + 1) * 128],
                                     identb[:], is_transpose=True)
                    for h in range(H_LOC):
                        base = (h * NT + jt) * 65
                        if b == 0 and jt < 8:
                            nc.scalar.copy(vn[b][:, base:base + 64],
                                           pv[:, h * 64:h * 64 + 64])
                        else:
                            nc.vector.tensor_copy(vn[b][:, base:base + 64],
                                                  pv[:, h * 64:h * 64 + 64])
                    if jt % 4 == 3:
                        yield

            def gen_wo(b, irp, evac_dve=True):
                # per-(half, ec) steps, half-major; stage-pipelined evac;
                # one output DMA per ec (fires after the half-1 evacuation)
                pend = None
                osb_tiles = {}

                def evac(p):
                    pw, ec, half, i = p
                    if ec not in osb_tiles:
                        osb_tiles[ec] = outsp.tile(
                            [128, 1024], bf16, tag="osb",
                            name=f"osb{b}_{ec}_{irp}")
                    osb = osb_tiles[ec]
                    dst = osb[:, half * 512:(half + 1) * 512]
                    if evac_dve is True or (evac_dve == "alt" and (ec + half) % 2 == 0):
                        nc.vector.tensor_copy(dst, pw[:])
                    else:
                        nc.scalar.copy(dst, pw[:])
                    if half == 1:
                        nc.sync.dma_start(
                            out=o[b, ec * 128:(ec + 1) * 128,
                                  irp * 1024:(irp + 1) * 1024],
                            in_=osb[:])

                i = 0
                for half in range(2):
                    for ec in range(NC_CHUNK):
                        pw = psD.tile([128, 512], f32, tag="mm",
                                      name=f"pw{b}_{ec}_{irp}_{half}")
                        nc.tensor.matmul(
                            pw[:],
                            wo_sb[:, ec * 128:(ec + 1) * 128],
                            o2[b][:, irp * 1024 + half * 512:
                                  irp * 1024 + (half + 1) * 512],
                            start=True, stop=True)
                        if pend is not None:
                            evac(pend)
                        pend = (pw, ec, half, i)
                        i += 1
                        yield
                evac(pend)

            eb_tiles = {}

            def emit_eb_ir(h, ir):
                if (h, ir) in eb_tiles:
                    return
                njt = min(4 * ir + 4, NT)
                t = ebpool.tile([128, njt * 512], bf16, tag=f"eb{ir}", bufs=2,
                                name=f"eb_{h}_{ir}")
                nc.sync.dma_start(
                    out=t[:].rearrange("p (j f) -> p j f", j=njt),
                    in_=eb[h, 0:njt * 128, ir * 512:(ir + 1) * 512].rearrange(
                        "(j p) f -> p j f", j=njt))
                eb_tiles[(h, ir)] = t

            def emit_eb_tile(h, jtp, ir):
                emit_eb_ir(h, ir)

            def gen_eb(h):
                for ir in range(NIR):
                    emit_eb_ir(h, ir)
                    yield

            def gen_attn(h, b):
                # stage-pipelined: PV (+ epilogue at ir close) for block k-1
                # is emitted after QK/exp/mul of block k, so no engine queue
                # entry ever waits long (avoids head-of-line blocking).
                pends = []

                def epilogue(po, ir):
                    rfl = dancep.tile([1, 512], bf16, tag="rfl", bufs=2,
                                      name=f"rf{h}_{b}_{ir}")
                    with nc.allow_low_precision(reason="bf16 1/denom"):
                        nc.vector.reciprocal(rfl[0:1, :], po[64:65, :])
                    rb = ework.tile([128, 512], bf16, tag="rb", bufs=2,
                                    name=f"rb{h}_{b}_{ir}")
                    for s in range(4):
                        nc.gpsimd.partition_broadcast(
                            rb[:, s * 128:(s + 1) * 128],
                            rfl[0:1, s * 128:(s + 1) * 128])
                    if h == 0:
                        nc.vector.tensor_mul(
                            o2[b][0:64, ir * 512:(ir + 1) * 512],
                            po[0:64, :], rb[0:64, :])
                    else:
                        tmp = ework.tile([64, 512], bf16, tag="hshift",
                                         bufs=2, name=f"hs{h}_{b}_{ir}")
                        nc.vector.tensor_mul(tmp[:], po[0:64, :], rb[0:64, :])
                        nc.sync.dma_start(
                            out=o2[b][64:128, ir * 512:(ir + 1) * 512],
                            in_=tmp[:])

                def stage2(p):
                    e2, jtp, jt_max, po, ir = p
                    for half in range(2):
                        jt = 2 * jtp + half
                        vbase = (h * NT + jt) * 65
                        nc.tensor.matmul(
                            po[:], vn[b][:, vbase:vbase + 65],
                            e2[:, half * 512:(half + 1) * 512],
                            start=(jt == 0), stop=(jt == jt_max - 1))
                    if 2 * jtp + 1 == jt_max - 1:
                        epilogue(po, ir)

                for ir in range(NIR):
                    jt_max = min(4 * ir + 4, NT)
                    po = psB.tile([65, 512], f32, tag="po",
                                  name=f"po{h}_{b}_{ir}")
                    for jtp in range(jt_max // 2):
                        emit_eb_ir(h, ir)
                        ebt = eb_tiles[(h, ir)][:, jtp * 1024:(jtp + 1) * 1024]
                        # last jtp of each ir is the diagonal pair: j > i for
                        # all i in the first half of the i-range, so compute
                        # only i in [256, 512) and zero-fill the rest of e2
                        diag = (jtp == jt_max // 2 - 1)
                        i0 = 256 if diag else 0
                        ps = psA.tile([128, 1024], f32, tag="qk",
                                      name=f"ps{h}_{b}_{ir}_{jtp}")
                        for half in range(2):
                            jt = 2 * jtp + half
                            nc.tensor.matmul(
                                ps[:, half * 512 + i0:(half + 1) * 512],
                                kT[b][h * 64:(h + 1) * 64,
                                      jt * 128:(jt + 1) * 128],
                                qT[b][h * 64:(h + 1) * 64,
                                      ir * 512 + i0:(ir + 1) * 512],
                                start=True, stop=True)
                        e1 = ework.tile([128, 1024], bf16, tag="e1", bufs=6,
                                        name=f"e1_{h}_{b}_{ir}_{jtp}")
                        e2 = ework.tile([128, 1024], bf16, tag="e2", bufs=6,
                                        name=f"e2_{h}_{b}_{ir}_{jtp}")
                        if diag:
                            ps3 = ps[:].rearrange("p (h f) -> p h f", h=2)
                            e13 = e1[:].rearrange("p (h f) -> p h f", h=2)
                            e23 = e2[:].rearrange("p (h f) -> p h f", h=2)
                            eb3 = ebt.rearrange("p (h f) -> p h f", h=2)
                            nc.gpsimd.memset(e23[:, :, 0:256], 0.0)
                            nc.scalar.activation(
                                e13[:, :, 256:512], ps3[:, :, 256:512],
                                mybir.ActivationFunctionType.Exp,
                                scale=1.0 / 1024.0)
                            nc.vector.tensor_mul(e23[:, :, 256:512],
                                                 e13[:, :, 256:512],
                                                 eb3[:, :, 256:512])
                        else:
                            nc.scalar.activation(
                                e1[:], ps[:],
                                mybir.ActivationFunctionType.Exp,
                                scale=1.0 / 1024.0)
                            nc.vector.tensor_mul(e2[:], e1[:], ebt)
                        pends.append((e2, jtp, jt_max, po, ir))
                        if len(pends) > 1:
                            stage2(pends.pop(0))
                        yield
                for p_ in pends:
                    stage2(p_)
                pends.clear()

            def run(g):
                for _ in g:
                    pass

            def take(g, n):
                for _ in range(n):
                    try:
                        next(g)
                    except StopIteration:
                        return False
                return True

            def rr2(main, other, ratio=2):
                """Interleave ratio:1; stop when main exhausts (other may
                have leftovers for a later phase)."""
                while True:
                    for _ in range(ratio):
                        if not take(main, 1):
                            return
                    take(other, 1)

            def chain_steps(steps):
                for g, n in steps:
                    for _ in range(n):
                        try:
                            yield next(g)
                        except StopIteration:
                            break
                for g, _ in steps:
                    yield from g

            def zip_gens(*pairs):
                """Round-robin over (gen, per-slot count); yield per round."""
                active = [[g, c] for g, c in pairs]
                while active:
                    for it in list(active):
                        for _ in range(it[1]):
                            try:
                                next(it[0])
                            except StopIteration:
                                active.remove(it)
                                break
                        yield

            def pair(ga, gb):
                """Interleave two attention streams, 2 blocks per turn;
                stream A primed 2 blocks ahead so the per-ir epilogue
                bursts of the two streams don't collide."""
                take(ga, 2)
                while True:
                    a_ok = take(ga, 2)
                    b_ok = take(gb, 2)
                    if not (a_ok or b_ok):
                        return
                    yield
                    yield

            p0 = gen_proj(0)
            vt0 = gen_vt(0)
            p1 = gen_proj(1, evac_dve=True)
            v1 = gen_vt(1)
            ebz = chain_steps([(gen_eb(0), 100), (gen_eb(1), 100)])

            # intro: proj-b0 irp0 (v,k,q) + vt jt0-7, eb-h0 trickling on DMA
            for _ in range(3):
                take(p0, 1)
                take(ebz, 1)
            take(vt0, 1)

            # phase C: both b0 attention streams paired, over the rest of
            # b0-proj, all of b1-proj, and wo(b0,0) once deps complete
            paC = pair(gen_attn(0, 0), gen_attn(1, 0))
            bgC = chain_steps([(vt0, 1), (p0, 3), (vt0, 2), (p1, 6), (v1, 4)])
            for _ in range(7):
                take(paC, 1)
                take(bgC, 1)
            w00 = gen_wo(0, 0, evac_dve="alt")
            bgC2 = chain_steps([(bgC, 999), (w00, 100), (ebz, 999)])
            rr2(paC, bgC2, 1)
            run(bgC)   # safety barrier: all projections emitted
            run(w00)

            # phase E: both b1 attention streams over remaining wo work.
            paE = pair(gen_attn(0, 1), gen_attn(1, 1))
            w01 = gen_wo(0, 1, evac_dve="alt")    # deps met (phase C done)
            ww = chain_steps([(w01, 100)])
            for _ in range(14):
                take(paE, 1)
                take(ww, 1)
                if _ >= 7 and _ % 2 == 1:
                    take(ww, 1)
            run(w01)
            w10 = gen_wo(1, 0, evac_dve="alt")    # ir0-1 epilogues done
            for _ in range(4):
                take(paE, 1)
                take(w10, 4)
            run(w10)
            w11 = gen_wo(1, 1, evac_dve="alt")    # half0 needs ir2 (round ~14)
            rr2(paE, chain_steps([(w11, 8), (ebz, 999)]), 1)
            run(w11)

    nc.compile()
    return nc


def _get_nc():
    if "nc" not in _BUILT:
        _BUILT["nc"] = _build()
    return _BUILT["nc"]


def _reference_np(x, attn_bias, mask, gamma, Wq, Wkv, Wo):
    """Numpy fallback (only used if mask is not all-True)."""
    b, n, dim = x.shape
    h, dh = HEADS, DH
    l2 = np.sqrt((x.astype(np.float64) ** 2).sum(-1, keepdims=True))
    xn = x / np.maximum(l2, 1e-12) * (dim ** 0.5) * gamma
    q = (xn @ Wq.T) * (dh ** -0.5)
    kv = xn @ Wkv.T
    k, v = kv[..., :h * dh], kv[..., h * dh:]
    def to_heads(t):
        return t.reshape(b, n, h, dh).transpose(0, 2, 1, 3)
    q, k, v = to_heads(q), to_heads(k), to_heads(v)
    sim = np.einsum('bhid,bhjd->bhij', q, k) + attn_bias
    neg = -np.finfo(np.float32).max
    sim = np.where(mask[:, None, None, :], sim, neg)
    causal = np.triu(np.ones((n, n), dtype=bool), k=1)
    sim = np.where(causal, neg, sim)
    sim = sim - sim.max(-1, keepdims=True)
    e = np.exp(sim)
    attn = e / e.sum(-1, keepdims=True)
    out = np.einsum('bhij,bhjd->bhid', attn, v)
    out = out.transpose(0, 2, 1, 3).reshape(b, n, h * dh)
    return (out @ Wo.T).astype(np.float32)


def kernel(x, attn_bias, mask, gamma, Wq, Wkv, Wo, _trace=False):
    from concourse.bass_utils import run_bass_kernel_spmd

    x = np.asarray(x, dtype=np.float32)
    attn_bias = np.asarray(attn_bias, dtype=np.float32)
    mask = np.asarray(mask)
    gamma = np.asarray(gamma, dtype=np.float32)
    Wq = np.asarray(Wq, dtype=np.float32)
    Wkv = np.asarray(Wkv, dtype=np.float32)
    Wo = np.asarray(Wo, dtype=np.float32)

    if not bool(mask.all()):
        return _reference_np(x, attn_bias, mask, gamma, Wq, Wkv, Wo)

    # ---- host prep (elementwise / layout only) ----
    bf = ml_dtypes.bfloat16
    f8 = ml_dtypes.float8_e4m3
    WS = 32.0   # fp8 weight scale; q,k each carry x32 -> exp scale 1/1024,
                # v carries x32 -> folded into Wo below
    l2 = np.sqrt((x ** 2).sum(-1, keepdims=True))
    xn = x / np.maximum(l2, 1e-12) * (DIM ** 0.5) * gamma
    xnT = np.ascontiguousarray(xn.transpose(0, 2, 1))            # [B, DIM, N]
    xnT8 = xnT.astype(f8)
    xnTc = np.ascontiguousarray(xnT[:, :, :128]).astype(bf)

    tril = np.tril(np.ones((N, N), dtype=np.float32))

    def f8_layout(wT):
        """[DIM,128] -> [128, DIM] DoubleRow image:
        out[p, ((g*2 + kt)*128) + j] = wT[g*256 + kt*128 + p, j]."""
        return np.ascontiguousarray(
            wT.reshape(4, 2, 128, 128).transpose(2, 0, 1, 3).reshape(128, DIM))

    def sbuf_layout(wT):
        """[DIM,128] -> [128, DIM] SBUF image: out[p, c*128+j] = wT[c*128+p, j]."""
        return np.ascontiguousarray(
            wT.reshape(NC_CHUNK, 128, 128).transpose(1, 0, 2).reshape(128, DIM))

    in_maps = []
    for c in range(N_CORES):
        r0 = c * 128
        wq_c = f8_layout((Wq[r0:r0 + 128] * (WS * DH ** -0.5)).T).astype(f8)
        wk_c = f8_layout((Wkv[r0:r0 + 128] * WS).T).astype(f8)
        wv_c = f8_layout(
            (Wkv[HEADS * DH + r0:HEADS * DH + r0 + 128] * WS).T).astype(f8)
        wvc_c = sbuf_layout(
            (Wkv[HEADS * DH + r0:HEADS * DH + r0 + 128] * WS).T).astype(bf)
        wo_c = np.ascontiguousarray(
            (Wo[:, r0:r0 + 128] / WS).T).astype(bf)  # [128, DIM]
        bias_c = attn_bias[H_LOC * c:H_LOC * (c + 1)]            # [2, N, N]
        ebc = np.exp(bias_c) * tril                              # mask j>i
        ebc = np.ascontiguousarray(ebc.transpose(0, 2, 1)).astype(bf)  # [h,j,i]
        in_maps.append({"xnT": xnT8, "xnTc": xnTc, "wq": wq_c, "wk": wk_c,
                        "wv": wv_c, "wvc": wvc_c, "wo": wo_c, "eb": ebc})

    nc = _get_nc()
    try:
        res = run_bass_kernel_spmd(nc, in_maps,
                                   core_ids=list(range(N_CORES)),
                                   trace=_trace)
    except ModuleNotFoundError:
        # NTFF profiling hook unavailable in this environment
        res = run_bass_kernel_spmd(nc, in_maps,
                                   core_ids=list(range(N_CORES)))
    acc = res.results[0]["o"].astype(np.float32)
    for c in range(1, N_CORES):
        acc += res.results[c]["o"].astype(np.float32)
    out = np.ascontiguousarray(acc.transpose(0, 2, 1))           # [B, N, DIM]
    if _trace:
        kernel._last_results = res
    return out
